# revision 1
# baseline (speedup 1.0000x reference)
"""MoE feed-forward (8 experts, top-2, 2 shared experts) on 8 TRN2 NeuronCores.

Strategy (expert-parallel):
  - 1 expert per core. Router computed on-device per core for its own 1/8
    token slice (token-major), then a tiny AllToAll distributes comb columns
    so core e ends up with gate weights of expert e for ALL tokens.
  - Dense expert FFN per core in fp32r (stage1, feature-major hidden) /
    bf16 (stage2, token-major output). Gate scaling is a per-partition
    tensor_scalar on the token-major output.
  - ReduceScatter sums expert contributions across cores and hands each
    core its own token slice.
  - Shared experts are computed data-parallel (each core: its token slice),
    overlapping the ReduceScatter, and added locally before writing out.
"""

import sys

if "/opt/trn_rl_repo" not in sys.path:
    sys.path.insert(0, "/opt/trn_rl_repo")

import numpy as np
import ml_dtypes

import concourse.bass as bass
import concourse.tile as tile
import concourse.mybir as mybir
from concourse import bacc
from concourse.bass_utils import run_bass_kernel_spmd

F32 = mybir.dt.float32
F32R = mybir.dt.float32r
BF16 = mybir.dt.bfloat16
I32 = mybir.dt.int32
AX = mybir.AxisListType
ALU = mybir.AluOpType
ACTF = mybir.ActivationFunctionType

N, D, HE, E, S = 8192, 1024, 2048, 8, 2
NCORES = 8
NT = N // NCORES      # 1024 tokens per core slice
TBC = 1024            # token chunk for the dense expert stages
NTBC = N // TBC
ND = D // 128         # 8
NH = HE // 128        # 16
RG = [list(range(NCORES))]

CAP = 2560            # sparse: per-expert token capacity (mean load 2048)
CB = CAP // 128       # 20 slot blocks
CF = CAP // 16        # sparse_gather output free dim
SWEEPS = (8, 8, 4)    # slot blocks per sweep (1024/1024/512 slots)

_NC_CACHE = {}


def _build(has_rb, has_b2, has_sb2, debug=False):
    nc = bacc.Bacc(None, target_bir_lowering=False)

    xt_p = nc.declare_dram_parameter("xt", [D, N], F32R, isOutput=False)
    xsl_p = nc.declare_dram_parameter("xsl", [D, NT], F32R, isOutput=False)
    w1_p = nc.declare_dram_parameter("w1", [D, HE], F32R, isOutput=False)
    w2_p = nc.declare_dram_parameter("w2", [HE, D], BF16, isOutput=False)
    rw_p = nc.declare_dram_parameter("rw", [128, ND, E], F32R, isOutput=False)
    sw1_p = nc.declare_dram_parameter("sw1", [S * D, HE], F32R, isOutput=False)
    sw2_p = nc.declare_dram_parameter("sw2", [S * HE, D], BF16, isOutput=False)
    b1v_p = nc.declare_dram_parameter("b1v", [128, NH], F32, isOutput=False)
    sb1v_p = nc.declare_dram_parameter("sb1v", [128, S * NH], F32, isOutput=False)
    id_p = nc.declare_dram_parameter("ident", [128, 128], F32, isOutput=False)
    if has_rb:
        rb_p = nc.declare_dram_parameter("rbr", [128, E], F32, isOutput=False)
    if has_b2:
        b2_p = nc.declare_dram_parameter("b2r", [128, D], F32, isOutput=False)
    if has_sb2:
        sb2_p = nc.declare_dram_parameter("sb2r", [128, D], F32, isOutput=False)
    yo_p = nc.declare_dram_parameter("y_out", [NT, D], F32, isOutput=True)
    if debug:
        dbg_combT_p = nc.declare_dram_parameter("dbg_combT", [E, NT], F32, isOutput=True)
        dbg_ctm_p = nc.declare_dram_parameter("dbg_ctm", [128, N // 128], F32, isOutput=True)
        dbg_ysh_p = nc.declare_dram_parameter("dbg_ysh", [NT, D], F32, isOutput=True)

    from contextlib import ExitStack

    with tile.TileContext(nc) as tc, ExitStack() as ctx:
        ep = ctx.enter_context
        dram = ep(tc.tile_pool(name="dram", bufs=1, space="DRAM"))
        cpool = ep(tc.tile_pool(name="cpool", bufs=1))
        xslp = ep(tc.tile_pool(name="xslp", bufs=1))
        xtp = ep(tc.tile_pool(name="xtp", bufs=1))
        htp = ep(tc.tile_pool(name="htp", bufs=1))
        w2rp = ep(tc.tile_pool(name="w2rp", bufs=1))
        wst = ep(tc.tile_pool(name="wst", bufs=6))
        sw2st = ep(tc.tile_pool(name="sw2st", bufs=3))
        ysbp = ep(tc.tile_pool(name="ysbp", bufs=2))
        finp = ep(tc.tile_pool(name="finp", bufs=2))
        rp = ep(tc.tile_pool(name="rp", bufs=2))
        ps1 = ep(tc.tile_pool(name="ps1", bufs=2, space="PSUM"))
        ps2 = ep(tc.tile_pool(name="ps2", bufs=2, space="PSUM"))

        moe_y = dram.tile([N, D], F32, name="moe_y")
        rs_out = dram.tile([NT, D], F32, name="rs_out")
        a2a_in = dram.tile([E, NT], F32, name="a2a_in")
        a2a_out = dram.tile([E, NT], F32, name="a2a_out")

        ident = cpool.tile([128, 128], F32, name="ident")
        nc.sync.dma_start(ident[:], id_p[:])
        b1v = cpool.tile([128, NH], F32, name="b1v")
        nc.sync.dma_start(b1v[:], b1v_p[:])
        sb1v = cpool.tile([128, S * NH], F32, name="sb1v")
        nc.sync.dma_start(sb1v[:], sb1v_p[:])
        # router path in true fp32 tiles (PE matmul mode follows the SBUF
        # tensor dtype; fp32r noise ~3e-4 exceeds the smallest gate gap 2e-5)
        rw_sb = cpool.tile([128, ND, E], F32, name="rw_sb")
        nc.sync.dma_start(rw_sb[:], rw_p[:].bitcast(F32))
        if has_rb:
            rbr = cpool.tile([128, E], F32, name="rbr")
            nc.sync.dma_start(rbr[:], rb_p[:])
        if has_b2:
            b2r = cpool.tile([128, D], F32, name="b2r")
            nc.sync.dma_start(b2r[:], b2_p[:])
        if has_sb2:
            sb2r = cpool.tile([128, D], F32, name="sb2r")
            nc.sync.dma_start(sb2r[:], sb2_p[:])

        xsl = []
        for d in range(ND):
            t = xslp.tile([128, NT], F32R, tag=f"xsl{d}", name=f"xsl{d}")
            nc.sync.dma_start(t[:], xsl_p[d * 128 : (d + 1) * 128, :])
            xsl.append(t)

        # ---------------- router (own token slice, token-major) ----------
        combT = cpool.tile([E, NT], F32, name="combT")
        for j in range(NT // 128):
            pg = ps1.tile([128, E], F32, tag="hpsum", name=f"pg{j}")
            for d in range(ND):
                xr = rp.tile([128, 128], F32, tag="xr", name=f"xr{j}_{d}", bufs=4)
                nc.sync.dma_start(
                    xr[:],
                    xsl_p[d * 128 : (d + 1) * 128, j * 128 : (j + 1) * 128].bitcast(F32),
                )
                nc.tensor.matmul(
                    pg[:],
                    xr[:],
                    rw_sb[:, d, :],
                    start=(d == 0),
                    stop=(d == ND - 1),
                )
            gates = rp.tile([128, E], F32, tag="gates", name=f"gates{j}")
            if has_rb:
                nc.vector.tensor_tensor(gates[:], pg[:], rbr[:], op=ALU.add)
            else:
                nc.vector.tensor_copy(gates[:], pg[:])
            m1 = rp.tile([128, 1], F32, tag="m1", name=f"m1_{j}")
            nc.vector.tensor_reduce(m1[:], gates[:], axis=AX.X, op=ALU.max)
            mask1 = rp.tile([128, E], F32, tag="mask1", name=f"mask1_{j}")
            nc.vector.tensor_scalar(mask1[:], gates[:], m1[:], None, op0=ALU.is_equal)
            negm = rp.tile([128, E], F32, tag="negm", name=f"negm{j}")
            nc.vector.tensor_scalar(negm[:], mask1[:], -1e30, None, op0=ALU.mult)
            gm = rp.tile([128, E], F32, tag="gm", name=f"gm{j}")
            nc.vector.tensor_tensor(gm[:], gates[:], negm[:], op=ALU.add)
            m2 = rp.tile([128, 1], F32, tag="m2", name=f"m2_{j}")
            nc.vector.tensor_reduce(m2[:], gm[:], axis=AX.X, op=ALU.max)
            mask2 = rp.tile([128, E], F32, tag="mask2", name=f"mask2_{j}")
            nc.vector.tensor_scalar(mask2[:], gm[:], m2[:], None, op0=ALU.is_equal)
            dl = rp.tile([128, 1], F32, tag="dl", name=f"dl{j}")
            nc.vector.tensor_tensor(dl[:], m2[:], m1[:], op=ALU.subtract)
            e2 = rp.tile([128, 1], F32, tag="e2", name=f"e2_{j}")
            nc.scalar.activation(e2[:], dl[:], ACTF.Exp)
            den = rp.tile([128, 1], F32, tag="den", name=f"den{j}")
            nc.vector.tensor_scalar_add(den[:], e2[:], 1.0)
            p1 = rp.tile([128, 1], F32, tag="p1", name=f"p1_{j}")
            nc.vector.reciprocal(p1[:], den[:])
            p2 = rp.tile([128, 1], F32, tag="p2", name=f"p2_{j}")
            nc.vector.tensor_tensor(p2[:], e2[:], p1[:], op=ALU.mult)
            t1 = rp.tile([128, E], F32, tag="t1", name=f"t1_{j}")
            nc.vector.tensor_scalar(t1[:], mask1[:], p1[:], None, op0=ALU.mult)
            t2 = rp.tile([128, E], F32, tag="t2", name=f"t2_{j}")
            nc.vector.tensor_scalar(t2[:], mask2[:], p2[:], None, op0=ALU.mult)
            cj = rp.tile([128, E], F32, tag="cj", name=f"cj{j}")
            nc.vector.tensor_tensor(cj[:], t1[:], t2[:], op=ALU.add)
            pt = ps1.tile([E, 128], F32, tag="hpsum", name=f"pt{j}")
            nc.tensor.transpose(pt[:], cj[:], ident[:])
            nc.vector.tensor_copy(combT[:, j * 128 : (j + 1) * 128], pt[:])
        nc.sync.dma_start(a2a_in[:], combT[:])
        nc.gpsimd.collective_compute(
            "AllToAll",
            ALU.bypass,
            replica_groups=RG,
            ins=[a2a_in.opt()],
            outs=[a2a_out.opt()],
        )
        comb_tm = cpool.tile([128, N // 128], F32, name="comb_tm")
        nc.sync.dma_start(
            comb_tm[:], a2a_out[:].rearrange("a (c p) -> p (a c)", p=128)
        )
        if debug:
            nc.sync.dma_start(dbg_combT_p[:], combT[:])
            nc.sync.dma_start(dbg_ctm_p[:], comb_tm[:])

        # ---------------- dense expert FFN ------------------------------
        w2res = []
        for h in range(NH):
            t = w2rp.tile([128, D], BF16, tag=f"w2r{h}", name=f"w2r{h}")
            nc.sync.dma_start(t[:], w2_p[h * 128 : (h + 1) * 128, :])
            w2res.append(t)

        for tb in range(NTBC):
            xts = []
            for d in range(ND):
                t = xtp.tile([128, TBC], F32R, tag=f"xt{d}", name=f"xt{tb}_{d}")
                nc.sync.dma_start(t[:], xt_p[d * 128 : (d + 1) * 128, tb * TBC : (tb + 1) * TBC])
                xts.append(t)
            hts = []
            for h in range(NH):
                ph = ps1.tile([128, TBC], F32, tag="hpsum", name=f"ph{tb}_{h}")
                for d in range(ND):
                    w1t = wst.tile([128, 128], F32R, tag="w1t", name=f"w1t{tb}_{h}_{d}")
                    nc.sync.dma_start(w1t[:], w1_p[d * 128 : (d + 1) * 128, h * 128 : (h + 1) * 128])
                    for v in range(TBC // 512):
                        nc.tensor.matmul(
                            ph[:, v * 512 : (v + 1) * 512],
                            w1t[:],
                            xts[d][:, v * 512 : (v + 1) * 512],
                            start=(d == 0),
                            stop=(d == ND - 1),
                        )
                ht = htp.tile([128, TBC], BF16, tag=f"hT{h}", name=f"hT{tb}_{h}")
                nc.scalar.activation(ht[:], ph[:], ACTF.Gelu, bias=b1v[:, h : h + 1])
                hts.append(ht)
            for k in range(TBC // 128):
                g = tb * (TBC // 128) + k
                py = ps2.tile([128, D], F32, tag="ypsum", name=f"py{g}")
                for h in range(NH):
                    for v in range(D // 512):
                        nc.tensor.matmul(
                            py[:, v * 512 : (v + 1) * 512],
                            hts[h][:, k * 128 : (k + 1) * 128],
                            w2res[h][:, v * 512 : (v + 1) * 512],
                            start=(h == 0),
                            stop=(h == NH - 1),
                        )
                if has_b2:
                    nc.vector.tensor_tensor(py[:], py[:], b2r[:], op=ALU.add)
                ysb = ysbp.tile([128, D], F32, tag="ysb", name=f"ysb{g}")
                nc.vector.tensor_scalar(ysb[:], py[:], comb_tm[:, g : g + 1], None, op0=ALU.mult)
                nc.sync.dma_start(moe_y[g * 128 : (g + 1) * 128, :], ysb[:])

        # ---------------- combine across cores --------------------------
        nc.gpsimd.collective_compute(
            "ReduceScatter",
            ALU.add,
            replica_groups=RG,
            ins=[moe_y.opt()],
            outs=[rs_out.opt()],
        )

        # ---------------- shared experts (overlap the RS) ----------------
        ysh = []
        for k in range(NT // 128):
            t = xtp.tile([128, D], F32, tag=f"xt{k}", name=f"ysh{k}")
            ysh.append(t)
        for s in range(S):
            shts = []
            for h in range(NH):
                ph = ps1.tile([128, NT], F32, tag="hpsum", name=f"shp{s}_{h}")
                for d in range(ND):
                    swt = wst.tile([128, 128], F32R, tag="w1t", name=f"swt{s}_{h}_{d}")
                    nc.sync.dma_start(swt[:], sw1_p[s * D + d * 128 : s * D + (d + 1) * 128, h * 128 : (h + 1) * 128])
                    for v in range(NT // 512):
                        nc.tensor.matmul(
                            ph[:, v * 512 : (v + 1) * 512],
                            swt[:],
                            xsl[d][:, v * 512 : (v + 1) * 512],
                            start=(d == 0),
                            stop=(d == ND - 1),
                        )
                sht = htp.tile([128, NT], BF16, tag=f"hT{h}", name=f"shT{s}_{h}")
                nc.scalar.activation(sht[:], ph[:], ACTF.Gelu, bias=sb1v[:, s * NH + h : s * NH + h + 1])
                shts.append(sht)
            for kg in range(NT // 256):
                pys = []
                for ki in range(2):
                    k = kg * 2 + ki
                    pys.append(ps2.tile([128, D], F32, tag="ypsum", name=f"spy{s}_{k}"))
                for h in range(NH):
                    sw2t = sw2st.tile([128, D], BF16, tag="sw2t", name=f"sw2t{s}_{kg}_{h}")
                    nc.sync.dma_start(sw2t[:], sw2_p[s * HE + h * 128 : s * HE + (h + 1) * 128, :])
                    for ki in range(2):
                        k = kg * 2 + ki
                        for v in range(D // 512):
                            nc.tensor.matmul(
                                pys[ki][:, v * 512 : (v + 1) * 512],
                                shts[h][:, k * 128 : (k + 1) * 128],
                                sw2t[:, v * 512 : (v + 1) * 512],
                                start=(h == 0),
                                stop=(h == NH - 1),
                            )
                for ki in range(2):
                    k = kg * 2 + ki
                    if s == 0:
                        nc.vector.tensor_copy(ysh[k][:], pys[ki][:])
                    else:
                        nc.vector.tensor_tensor(ysh[k][:], ysh[k][:], pys[ki][:], op=ALU.add)

        # ---------------- final: rs slice + shared ------------------------
        for k in range(NT // 128):
            fin = finp.tile([128, D], F32, tag="fin", name=f"fin{k}")
            if debug:
                nc.sync.dma_start(dbg_ysh_p[k * 128 : (k + 1) * 128, :], ysh[k][:])
            nc.sync.dma_start(fin[:], rs_out[k * 128 : (k + 1) * 128, :])
            nc.vector.tensor_tensor(fin[:], fin[:], ysh[k][:], op=ALU.add)
            if has_sb2:
                nc.vector.tensor_tensor(fin[:], fin[:], sb2r[:], op=ALU.add)
            nc.sync.dma_start(yo_p[k * 128 : (k + 1) * 128, :], fin[:])

    nc.compile()
    return nc


def _build_sparse(has_rb, has_b2, has_sb2, debug=False):
    from concourse import library_config

    nc = bacc.Bacc(None, target_bir_lowering=False)

    NQ = 4                 # token quarters (chunked ReduceScatter)
    QTOK = N // NQ         # 2048 tokens per quarter
    CAPQ = 768             # per-(expert, quarter) slot capacity
    CBQ = CAPQ // 128      # 6 slot blocks per quarter
    CFQ = CAPQ // 16       # sparse_gather out free dim

    xtm_p = nc.declare_dram_parameter("xtm", [N, D], BF16, isOutput=False)
    xsl_p = nc.declare_dram_parameter("xsl", [D, NT], F32R, isOutput=False)
    w1_p = nc.declare_dram_parameter("w1", [D, HE], BF16, isOutput=False)
    w2_p = nc.declare_dram_parameter("w2", [HE, D], BF16, isOutput=False)
    rw_p = nc.declare_dram_parameter("rw", [128, ND, E], F32, isOutput=False)
    sw1_p = nc.declare_dram_parameter("sw1", [S * D, HE], F32R, isOutput=False)
    sw2_p = nc.declare_dram_parameter("sw2", [S * HE, D], BF16, isOutput=False)
    b1v_p = nc.declare_dram_parameter("b1v", [128, NH], F32, isOutput=False)
    sb1v_p = nc.declare_dram_parameter("sb1v", [128, S * NH], F32, isOutput=False)
    id_p = nc.declare_dram_parameter("ident", [128, 128], F32, isOutput=False)
    io16_p = nc.declare_dram_parameter("iota16", [16, N // 16], F32, isOutput=False)
    slio_p = nc.declare_dram_parameter("slotio", [128, CBQ], F32, isOutput=False)
    if has_rb:
        rb_p = nc.declare_dram_parameter("rbr", [128, E], F32, isOutput=False)
    if has_b2:
        b2_p = nc.declare_dram_parameter("b2r", [128, D], F32, isOutput=False)
    if has_sb2:
        sb2_p = nc.declare_dram_parameter("sb2r", [128, D], F32, isOutput=False)
    yo_p = nc.declare_dram_parameter("y_out", [NT, D], F32, isOutput=True)
    nf_p = nc.declare_dram_parameter("nf_out", [NQ], mybir.dt.uint32, isOutput=True)

    from contextlib import ExitStack

    with tile.TileContext(nc) as tc, ExitStack() as ctx:
        ep = ctx.enter_context
        dram = ep(tc.tile_pool(name="dram", bufs=1, space="DRAM"))
        cpool = ep(tc.tile_pool(name="cpool", bufs=1))
        xslp = ep(tc.tile_pool(name="xslp", bufs=1))
        xgp = ep(tc.tile_pool(name="xgp", bufs=13))
        xgtp = ep(tc.tile_pool(name="xgtp", bufs=1))
        htp = ep(tc.tile_pool(name="htp", bufs=1))
        w2rp = ep(tc.tile_pool(name="w2rp", bufs=1))
        wst = ep(tc.tile_pool(name="wst", bufs=6))
        sw2st = ep(tc.tile_pool(name="sw2st", bufs=3))
        ysbp = ep(tc.tile_pool(name="ysbp", bufs=3))
        rp = ep(tc.tile_pool(name="rp", bufs=2))
        cmp_ = ep(tc.tile_pool(name="cmp", bufs=1))
        ps1 = ep(tc.tile_pool(name="ps1", bufs=2, space="PSUM"))
        ps2 = ep(tc.tile_pool(name="ps2", bufs=2, space="PSUM"))

        nc.gpsimd.load_library(library_config.sparse_gather)

        moe_q = [dram.tile([QTOK + 128, D], F32, name=f"moe_q{q}") for q in range(NQ)]
        rs_q = [dram.tile([QTOK // NCORES, D], F32, name=f"rs_q{q}") for q in range(NQ)]
        a2a_in = dram.tile([E, NT], F32, name="a2a_in")
        a2a_out = dram.tile([E, NT], F32, name="a2a_out")
        idx_d = [dram.tile([CAPQ, 1], F32, name=f"idx_d{q}") for q in range(NQ)]
        gate_d = [dram.tile([CAPQ, 1], F32, name=f"gate_d{q}") for q in range(NQ)]
        shared_y = dram.tile([NT, D], F32, name="shared_y")

        ident = cpool.tile([128, 128], F32, name="ident")
        nc.sync.dma_start(ident[:], id_p[:])
        ident_bf = cpool.tile([128, 128], BF16, name="ident_bf")
        nc.vector.tensor_copy(ident_bf[:], ident[:])
        b1v = cpool.tile([128, NH], F32, name="b1v")
        nc.sync.dma_start(b1v[:], b1v_p[:])
        sb1v = cpool.tile([128, S * NH], F32, name="sb1v")
        nc.sync.dma_start(sb1v[:], sb1v_p[:])
        rw_sb = cpool.tile([128, ND, E], F32, name="rw_sb")
        nc.sync.dma_start(rw_sb[:], rw_p[:])
        slio = cpool.tile([128, CBQ], F32, name="slio")
        nc.sync.dma_start(slio[:], slio_p[:])
        io16 = cpool.tile([16, N // 16], F32, name="io16")
        nc.sync.dma_start(io16[:], io16_p[:])
        ones1 = cpool.tile([1, 128], F32, name="ones1")
        nc.vector.memset(ones1[:], 1.0)
        if has_rb:
            rbr = cpool.tile([128, E], F32, name="rbr")
            nc.sync.dma_start(rbr[:], rb_p[:])
        if has_b2:
            b2r = cpool.tile([128, D], F32, name="b2r")
            nc.sync.dma_start(b2r[:], b2_p[:])
        if has_sb2:
            sb2r = cpool.tile([128, D], F32, name="sb2r")
            nc.sync.dma_start(sb2r[:], sb2_p[:])

        xsl = []
        for d in range(ND):
            t = xslp.tile([128, NT], F32R, tag=f"xsl{d}", name=f"xsl{d}")
            nc.sync.dma_start(t[:], xsl_p[d * 128 : (d + 1) * 128, :])
            xsl.append(t)

        # ---------------- router (own tokens, token-major) ---------------
        combT = cpool.tile([E, NT], F32, name="combT")
        for j in range(NT // 128):
            pg = ps1.tile([128, E], F32, tag="hpsum", name=f"pg{j}")
            for d in range(ND):
                xr = rp.tile([128, 128], F32, tag="xr", name=f"xr{j}_{d}", bufs=4)
                nc.sync.dma_start(
                    xr[:],
                    xsl_p[d * 128 : (d + 1) * 128, j * 128 : (j + 1) * 128].bitcast(F32),
                )
                nc.tensor.matmul(
                    pg[:], xr[:], rw_sb[:, d, :], start=(d == 0), stop=(d == ND - 1)
                )
            gates = rp.tile([128, E], F32, tag="gates", name=f"gates{j}")
            if has_rb:
                nc.vector.tensor_tensor(gates[:], pg[:], rbr[:], op=ALU.add)
            else:
                nc.vector.tensor_copy(gates[:], pg[:])
            m1 = rp.tile([128, 1], F32, tag="m1", name=f"m1_{j}")
            nc.vector.tensor_reduce(m1[:], gates[:], axis=AX.X, op=ALU.max)
            mask1 = rp.tile([128, E], F32, tag="mask1", name=f"mask1_{j}")
            nc.vector.tensor_scalar(mask1[:], gates[:], m1[:], None, op0=ALU.is_equal)
            negm = rp.tile([128, E], F32, tag="negm", name=f"negm{j}")
            nc.vector.tensor_scalar(negm[:], mask1[:], -1e30, None, op0=ALU.mult)
            gm = rp.tile([128, E], F32, tag="gm", name=f"gm{j}")
            nc.vector.tensor_tensor(gm[:], gates[:], negm[:], op=ALU.add)
            m2 = rp.tile([128, 1], F32, tag="m2", name=f"m2_{j}")
            nc.vector.tensor_reduce(m2[:], gm[:], axis=AX.X, op=ALU.max)
            mask2 = rp.tile([128, E], F32, tag="mask2", name=f"mask2_{j}")
            nc.vector.tensor_scalar(mask2[:], gm[:], m2[:], None, op0=ALU.is_equal)
            dl = rp.tile([128, 1], F32, tag="dl", name=f"dl{j}")
            nc.vector.tensor_tensor(dl[:], m2[:], m1[:], op=ALU.subtract)
            e2 = rp.tile([128, 1], F32, tag="e2", name=f"e2_{j}")
            nc.scalar.activation(e2[:], dl[:], ACTF.Exp)
            den = rp.tile([128, 1], F32, tag="den", name=f"den{j}")
            nc.vector.tensor_scalar_add(den[:], e2[:], 1.0)
            p1 = rp.tile([128, 1], F32, tag="p1", name=f"p1_{j}")
            nc.vector.reciprocal(p1[:], den[:])
            p2 = rp.tile([128, 1], F32, tag="p2", name=f"p2_{j}")
            nc.vector.tensor_tensor(p2[:], e2[:], p1[:], op=ALU.mult)
            t1 = rp.tile([128, E], F32, tag="t1", name=f"t1_{j}")
            nc.vector.tensor_scalar(t1[:], mask1[:], p1[:], None, op0=ALU.mult)
            t2 = rp.tile([128, E], F32, tag="t2", name=f"t2_{j}")
            nc.vector.tensor_scalar(t2[:], mask2[:], p2[:], None, op0=ALU.mult)
            cj = rp.tile([128, E], F32, tag="cj", name=f"cj{j}")
            nc.vector.tensor_tensor(cj[:], t1[:], t2[:], op=ALU.add)
            pt = ps1.tile([E, 128], F32, tag="hpsum", name=f"pt{j}")
            nc.tensor.transpose(pt[:], cj[:], ident[:])
            nc.vector.tensor_copy(combT[:, j * 128 : (j + 1) * 128], pt[:])
        nc.sync.dma_start(a2a_in[:], combT[:])
        nc.gpsimd.collective_compute(
            "AllToAll",
            ALU.bypass,
            replica_groups=RG,
            ins=[a2a_in.opt()],
            outs=[a2a_out.opt()],
        )

        # ---------------- shared experts pass s=0 (hides the A2A) --------
        def shared_pass(s):
            shts = []
            for h in range(NH):
                ph = ps1.tile([128, NT], F32, tag="hpsum", name=f"shp{s}_{h}")
                for d in range(ND):
                    swt = wst.tile([128, 128], F32R, tag="w1t", name=f"swt{s}_{h}_{d}")
                    nc.sync.dma_start(
                        swt[:],
                        sw1_p[
                            s * D + d * 128 : s * D + (d + 1) * 128,
                            h * 128 : (h + 1) * 128,
                        ],
                    )
                    for v in range(NT // 512):
                        nc.tensor.matmul(
                            ph[:, v * 512 : (v + 1) * 512],
                            swt[:],
                            xsl[d][:, v * 512 : (v + 1) * 512],
                            start=(d == 0),
                            stop=(d == ND - 1),
                        )
                sht = htp.tile([128, NT], BF16, tag=f"hT{h}", name=f"shT{s}_{h}")
                nc.scalar.activation(
                    sht[:], ph[:], ACTF.Gelu, bias=sb1v[:, s * NH + h : s * NH + h + 1]
                )
                shts.append(sht)
            for kg in range(NT // 256):
                pys = []
                for ki in range(2):
                    k = kg * 2 + ki
                    pys.append(ps2.tile([128, D], F32, tag="ypsum", name=f"spy{s}_{k}"))
                for h in range(NH):
                    sw2t = sw2st.tile([128, D], BF16, tag="sw2t", name=f"sw2t{s}_{kg}_{h}")
                    nc.sync.dma_start(
                        sw2t[:], sw2_p[s * HE + h * 128 : s * HE + (h + 1) * 128, :]
                    )
                    for ki in range(2):
                        k = kg * 2 + ki
                        for v in range(D // 512):
                            nc.tensor.matmul(
                                pys[ki][:, v * 512 : (v + 1) * 512],
                                shts[h][:, k * 128 : (k + 1) * 128],
                                sw2t[:, v * 512 : (v + 1) * 512],
                                start=(h == 0),
                                stop=(h == NH - 1),
                            )
                for ki in range(2):
                    k = kg * 2 + ki
                    ysb = ysbp.tile([128, D], F32, tag="ysb", name=f"shy{s}_{k}")
                    if s == 0:
                        nc.vector.tensor_copy(ysb[:], pys[ki][:])
                    else:
                        ld = ysbp.tile([128, D], F32, tag="ysb", name=f"shl{s}_{k}")
                        nc.sync.dma_start(ld[:], shared_y[k * 128 : (k + 1) * 128, :])
                        nc.vector.tensor_tensor(ysb[:], ld[:], pys[ki][:], op=ALU.add)
                    nc.sync.dma_start(shared_y[k * 128 : (k + 1) * 128, :], ysb[:])

        shared_pass(0)

        # zero-fill the scatter targets on the scalar engine's DMA queue so
        # they don't delay the latency-critical loads on the sync queue
        zt = cpool.tile([128, D], F32, name="zt")
        nc.vector.memset(zt[:], 0.0)
        for q in range(NQ):
            for r2 in range(QTOK // 128):
                nc.scalar.dma_start(moe_q[q][r2 * 128 : (r2 + 1) * 128, :], zt[:])

        # ---------------- per-quarter compaction -------------------------
        nf_all = cpool.tile([1, NQ], mybir.dt.uint32, name="nf_all")
        idx_sb = []
        idxL_sb = []
        gate_sb = []
        for q in range(NQ):
            c16 = cmp_.tile([16, QTOK // 16], F32, tag="c16", name=f"c16_{q}")
            for a in range(E):
                nc.sync.dma_start(
                    c16[:, a * 16 : (a + 1) * 16],
                    a2a_out[a, q * 256 : (q + 1) * 256].rearrange("(f p) -> p f", p=16),
                )
            msk = cmp_.tile([16, QTOK // 16], F32, tag="msk", name=f"msk{q}")
            nc.vector.tensor_scalar(msk[:], c16[:], 0.0, None, op0=ALU.not_equal)
            mm1 = cmp_.tile([16, QTOK // 16], F32, tag="mm1", name=f"mm1{q}")
            nc.vector.tensor_scalar(mm1[:], msk[:], 1.0, None, op0=ALU.subtract)
            # av = iota * mask + (mask-1): selected -> iota, unselected -> -1
            av = cmp_.tile([16, QTOK // 16], F32, tag="av", name=f"av{q}")
            nc.vector.tensor_tensor(
                av[:], io16[:, q * (QTOK // 16) : (q + 1) * (QTOK // 16)], msk[:], op=ALU.mult
            )
            nc.vector.tensor_tensor(av[:], av[:], mm1[:], op=ALU.add)
            # ag = comb + (mask-1): selected -> gate, unselected -> -1
            ag = cmp_.tile([16, QTOK // 16], F32, tag="ag", name=f"ag{q}")
            nc.vector.tensor_tensor(ag[:], c16[:], mm1[:], op=ALU.add)
            idxc = cmp_.tile([16, CFQ], F32, tag="idxc", name=f"idxc{q}")
            nc.vector.memset(idxc[:], 0.0)
            nfq = cmp_.tile([1, 1], mybir.dt.uint32, tag="nfq", name=f"nfq{q}")
            nc.gpsimd.sparse_gather(idxc[:], av[:], num_found=nfq[:])
            gatec = cmp_.tile([16, CFQ], F32, tag="gatec", name=f"gatec{q}")
            nc.vector.memset(gatec[:], 0.0)
            nfq2 = cmp_.tile([1, 1], mybir.dt.uint32, tag="nfq2", name=f"nfq2{q}")
            nc.gpsimd.sparse_gather(gatec[:], ag[:], num_found=nfq2[:])
            nc.vector.tensor_copy(nf_all[:, q : q + 1], nfq[:])
            # roundtrip to DRAM to relayout [16, CFQ] -> [128, CBQ]
            nc.sync.dma_start(
                idx_d[q][:].rearrange("(f p) one -> p (f one)", p=16), idxc[:]
            )
            nc.sync.dma_start(
                gate_d[q][:].rearrange("(f p) one -> p (f one)", p=16), gatec[:]
            )
            idxf = cmp_.tile([128, CBQ], F32, tag="idxf", name=f"idxf{q}")
            nc.sync.dma_start(
                idxf[:], idx_d[q][:].rearrange("(c p) one -> p (c one)", p=128)
            )
            gatef = cmp_.tile([128, CBQ], F32, tag="gatef", name=f"gatef{q}")
            nc.sync.dma_start(
                gatef[:], gate_d[q][:].rearrange("(c p) one -> p (c one)", p=128)
            )
            # tail mask from nf: slots >= nf get idx=N (skip), gate=0
            nff = cmp_.tile([1, 1], F32, tag="nff", name=f"nff{q}")
            nc.vector.tensor_copy(nff[:], nfq[:])
            nfb_ps = ps1.tile([128, 1], F32, tag="hpsum", name=f"nfb{q}")
            nc.tensor.matmul(nfb_ps[:], ones1[:], nff[:], start=True, stop=True)
            nfb = cmp_.tile([128, 1], F32, tag="nfb", name=f"nfbs{q}")
            nc.vector.tensor_copy(nfb[:], nfb_ps[:])
            mt = cmp_.tile([128, CBQ], F32, tag="mt", name=f"mt{q}")
            nc.vector.tensor_scalar(mt[:], slio[:], nfb[:], None, op0=ALU.is_ge)
            imt = cmp_.tile([128, CBQ], F32, tag="imt", name=f"imt{q}")
            nc.vector.tensor_scalar(imt[:], mt[:], -1.0, None, op0=ALU.mult)
            nc.vector.tensor_scalar(imt[:], imt[:], 1.0, None, op0=ALU.add)
            nc.vector.tensor_tensor(gatef[:], gatef[:], imt[:], op=ALU.mult)
            nc.vector.tensor_tensor(idxf[:], idxf[:], imt[:], op=ALU.mult)
            nc.vector.tensor_scalar(mt[:], mt[:], float(N), None, op0=ALU.mult)
            nc.vector.tensor_tensor(idxf[:], idxf[:], mt[:], op=ALU.add)
            # clamp for safety
            nc.vector.tensor_scalar(idxf[:], idxf[:], 0.0, None, op0=ALU.max)
            nc.vector.tensor_scalar(idxf[:], idxf[:], float(N), None, op0=ALU.min)
            ix = cmp_.tile([128, CBQ], I32, tag=f"ix{q}", name=f"ix{q}")
            nc.vector.tensor_copy(ix[:], idxf[:])
            # local (within-quarter) indices for the scatter
            ixl_f = cmp_.tile([128, CBQ], F32, tag="ixlf", name=f"ixlf{q}")
            nc.vector.tensor_scalar(ixl_f[:], idxf[:], float(q * QTOK), None, op0=ALU.subtract)
            ixl = cmp_.tile([128, CBQ], I32, tag=f"ixl{q}", name=f"ixl{q}")
            nc.vector.tensor_copy(ixl[:], ixl_f[:])
            gs = cmp_.tile([128, CBQ], F32, tag=f"gs{q}", name=f"gs{q}")
            nc.vector.tensor_copy(gs[:], gatef[:])
            idx_sb.append(ix)
            idxL_sb.append(ixl)
            gate_sb.append(gs)

        # ---------------- sparse expert FFN, one quarter at a time -------
        w2res = []
        for h in range(NH):
            t = w2rp.tile([128, D], BF16, tag=f"w2r{h}", name=f"w2r{h}")
            nc.sync.dma_start(t[:], w2_p[h * 128 : (h + 1) * 128, :])
            w2res.append(t)

        def emit_gathers(q):
            tiles = []
            for c in range(CBQ):
                xg = xgp.tile([128, D], BF16, tag="xg", name=f"xg{q}_{c}")
                nc.vector.memset(xg[:], 0.0)
                nc.gpsimd.indirect_dma_start(
                    out=xg[:],
                    out_offset=None,
                    in_=xtm_p[:],
                    in_offset=bass.IndirectOffsetOnAxis(
                        ap=idx_sb[q][:, c : c + 1], axis=0
                    ),
                    bounds_check=N - 1,
                    oob_is_err=False,
                )
                tiles.append(xg)
            return tiles

        xg_next = emit_gathers(0)
        for q in range(NQ):
            xg_cur = xg_next
            xgt = []
            for d in range(ND):
                t = xgtp.tile([128, CAPQ], BF16, tag=f"xgt{d}", name=f"xgt{q}_{d}")
                xgt.append(t)
            for c in range(CBQ):
                for d in range(ND):
                    tp = ps1.tile([128, 128], BF16, tag="hpsum", name=f"tp{q}_{c}_{d}")
                    nc.tensor.transpose(
                        tp[:], xg_cur[c][:, d * 128 : (d + 1) * 128], ident_bf[:]
                    )
                    nc.vector.tensor_copy(xgt[d][:, c * 128 : (c + 1) * 128], tp[:])
            # prefetch next quarter's gathers ahead of this quarter's
            # scatters in the gpsimd queue (avoids quarter serialization)
            if q + 1 < NQ:
                xg_next = emit_gathers(q + 1)
            hts = []
            for h in range(NH):
                ph = ps1.tile([128, CAPQ], F32, tag="hpsum", name=f"ph{q}_{h}")
                for d in range(ND):
                    w1t = wst.tile([128, 128], BF16, tag="w1b", name=f"w1t{q}_{h}_{d}")
                    nc.sync.dma_start(
                        w1t[:], w1_p[d * 128 : (d + 1) * 128, h * 128 : (h + 1) * 128]
                    )
                    nc.tensor.matmul(
                        ph[:, 0:512],
                        w1t[:],
                        xgt[d][:, 0:512],
                        start=(d == 0),
                        stop=(d == ND - 1),
                    )
                    nc.tensor.matmul(
                        ph[:, 512:CAPQ],
                        w1t[:],
                        xgt[d][:, 512:CAPQ],
                        start=(d == 0),
                        stop=(d == ND - 1),
                    )
                ht = htp.tile([128, CAPQ], BF16, tag=f"hT{h}", name=f"hTe{q}_{h}")
                nc.scalar.activation(ht[:], ph[:], ACTF.Gelu, bias=b1v[:, h : h + 1])
                hts.append(ht)
            for c in range(CBQ):
                py = ps2.tile([128, D], F32, tag="ypsum", name=f"py{q}_{c}")
                for h in range(NH):
                    for v in range(D // 512):
                        nc.tensor.matmul(
                            py[:, v * 512 : (v + 1) * 512],
                            hts[h][:, c * 128 : (c + 1) * 128],
                            w2res[h][:, v * 512 : (v + 1) * 512],
                            start=(h == 0),
                            stop=(h == NH - 1),
                        )
                if has_b2:
                    nc.vector.tensor_tensor(py[:], py[:], b2r[:], op=ALU.add)
                ysb = ysbp.tile([128, D], F32, tag="ysb", name=f"ysbq{q}_{c}")
                nc.vector.tensor_scalar(
                    ysb[:], py[:], gate_sb[q][:, c : c + 1], None, op0=ALU.mult
                )
                nc.gpsimd.indirect_dma_start(
                    out=moe_q[q][:],
                    out_offset=bass.IndirectOffsetOnAxis(
                        ap=idxL_sb[q][:, c : c + 1], axis=0
                    ),
                    in_=ysb[:],
                    in_offset=None,
                    bounds_check=QTOK + 127,
                    oob_is_err=False,
                )
            nc.gpsimd.collective_compute(
                "ReduceScatter",
                ALU.add,
                replica_groups=RG,
                ins=[moe_q[q][0:QTOK, :].opt()],
                outs=[rs_q[q].opt()],
            )

        # ---------------- shared experts pass s=1 (hides last RS) --------
        shared_pass(1)

        nc.sync.dma_start(nf_p[:], nf_all[:])

        # ---------------- final: rs quarters + shared --------------------
        for k in range(NT // 128):
            q, rr = k // 2, k % 2
            fin = ysbp.tile([128, D], F32, tag="ysb", name=f"fin{k}")
            nc.sync.dma_start(fin[:], rs_q[q][rr * 128 : (rr + 1) * 128, :])
            shl = ysbp.tile([128, D], F32, tag="ysb", name=f"finsh{k}")
            nc.sync.dma_start(shl[:], shared_y[k * 128 : (k + 1) * 128, :])
            nc.vector.tensor_tensor(fin[:], fin[:], shl[:], op=ALU.add)
            if has_sb2:
                nc.vector.tensor_tensor(fin[:], fin[:], sb2r[:], op=ALU.add)
            nc.sync.dma_start(yo_p[k * 128 : (k + 1) * 128, :], fin[:])

    nc.compile()
    return nc


def _get_nc(key):
    if key not in _NC_CACHE:
        _NC_CACHE[key] = _build_sparse(*key)
    return _NC_CACHE[key]


def _get_nc_dense(key):
    k2 = ("dense",) + key
    if k2 not in _NC_CACHE:
        _NC_CACHE[k2] = _build(*key)
    return _NC_CACHE[k2]


def _prep_in_maps(x, router_w, router_b, w1, b1, w2, b2, sw1, sb1, sw2, sb2,
                  sparse=True):
    f32 = np.float32
    x2 = np.ascontiguousarray(np.asarray(x, f32).reshape(N, D))
    x2bf = x2.astype(ml_dtypes.bfloat16)
    xt = np.ascontiguousarray(x2.T)

    has_rb = bool(np.any(router_b))
    has_b2 = bool(np.any(b2))
    sb2_eff = np.asarray(sb2, f32).sum(0) / S
    has_sb2 = bool(np.any(sb2_eff))
    key = (has_rb, has_b2, has_sb2)

    rw_r = np.ascontiguousarray(
        np.asarray(router_w, f32).reshape(ND, 128, E).transpose(1, 0, 2)
    )
    ident = np.eye(128, dtype=f32)
    # sparse variant: core e owns 256-token blocks {b : b % 8 == e}.
    # iota16[p, q*128 + a*16 + f] = global id of a2a-flat position
    # s' = a*256 + f*16 + p within quarter q = (a + 8q)*256 + f*16 + p
    iota16 = np.empty((16, N // 16), f32)
    for q in range(4):
        for a in range(E):
            for f in range(16):
                col = q * 128 + a * 16 + f
                iota16[:, col] = (a + 8 * q) * 256 + f * 16 + np.arange(16)
    slotio = (np.arange(6)[None, :] * 128 + np.arange(128)[:, None]).astype(f32)
    sw1s = np.ascontiguousarray(np.asarray(sw1, f32).reshape(S * D, HE))
    sw2s = np.ascontiguousarray(
        (np.asarray(sw2, f32) * (1.0 / S)).reshape(S * HE, D)
    ).astype(ml_dtypes.bfloat16)
    sb1v = np.ascontiguousarray(
        np.asarray(sb1, f32).reshape(S, NH, 128).transpose(2, 0, 1).reshape(128, S * NH)
    )
    if has_rb:
        rbr = np.tile(np.asarray(router_b, f32), (128, 1))
    if has_sb2:
        sb2r = np.tile(sb2_eff, (128, 1))

    in_maps = []
    for e in range(NCORES):
        if sparse:
            own = np.concatenate(
                [x2[(q * 8 + e) * 256 : (q * 8 + e + 1) * 256] for q in range(4)]
            )
            xsl_e = np.ascontiguousarray(own.T)
        else:
            xsl_e = np.ascontiguousarray(xt[:, e * NT : (e + 1) * NT])
        m = {
            "xsl": xsl_e,
            "w1": np.ascontiguousarray(np.asarray(w1[e], f32)),
            "w2": np.ascontiguousarray(np.asarray(w2[e], f32)).astype(ml_dtypes.bfloat16),
            "rw": rw_r,
            "sw1": sw1s,
            "sw2": sw2s,
            "b1v": np.ascontiguousarray(np.asarray(b1[e], f32).reshape(NH, 128).T),
            "sb1v": sb1v,
            "ident": ident,
        }
        if sparse:
            m["xtm"] = x2bf
            m["iota16"] = iota16
            m["slotio"] = slotio
            m["w1"] = m["w1"].astype(ml_dtypes.bfloat16)
        else:
            m["xt"] = xt
        if has_rb:
            m["rbr"] = rbr
        if has_b2:
            m["b2r"] = np.tile(np.asarray(b2[e], f32), (128, 1))
        if has_sb2:
            m["sb2r"] = sb2r
        in_maps.append(m)
    return key, in_maps


def _install_ntff_hook():
    """Re-create the boot-time NTFF profile hook (this image's antenv lacks
    axon_hooks, so trn_boot degraded silently). Needed only for tracing."""
    import contextlib
    import ctypes
    import types

    try:
        from antenv.axon_hooks import get_axon_ntff_profile_hook  # noqa: F401

        return
    except ImportError:
        pass

    so_path = "/opt/axon/libaxon_pjrt.so"
    lib = ctypes.CDLL(so_path)
    if not hasattr(lib, "axon_start_nrt_profile"):
        return
    lib.axon_start_nrt_profile.argtypes = [
        ctypes.POINTER(ctypes.c_int64),
        ctypes.c_size_t,
    ]
    lib.axon_start_nrt_profile.restype = ctypes.c_int64
    lib.axon_stop_nrt_profile.argtypes = [ctypes.c_char_p]
    lib.axon_stop_nrt_profile.restype = ctypes.c_int64

    @contextlib.contextmanager
    def _hook(output_dir, device_ids):
        import jax

        jax.devices()
        if device_ids:
            ids = (ctypes.c_int64 * len(device_ids))(*device_ids)
            rc = lib.axon_start_nrt_profile(ids, len(device_ids))
        else:
            rc = lib.axon_start_nrt_profile(None, 0)
        if rc != 0:
            raise RuntimeError(f"axon_start_nrt_profile rc={rc}")
        try:
            yield
        finally:
            n = lib.axon_stop_nrt_profile(str(output_dir).encode())
            print(f"profile: {n} file(s) written to {output_dir}", file=sys.stderr)

    mod = types.ModuleType("antenv.axon_hooks")
    mod.get_axon_ntff_profile_hook = lambda: _hook
    mod.set_axon_ntff_profile_hook = lambda h: None
    sys.modules["antenv.axon_hooks"] = mod


def kernel(x, router_w, router_b, w1, b1, w2, b2, sw1, sb1, sw2, sb2, _trace=False):
    if _trace:
        _install_ntff_hook()
    args = (x, router_w, router_b, w1, b1, w2, b2, sw1, sb1, sw2, sb2)
    key, in_maps = _prep_in_maps(*args, sparse=True)
    nc = _get_nc(key)
    res = run_bass_kernel_spmd(
        nc, in_maps, core_ids=list(range(NCORES)), trace=_trace
    )
    counts = [int(c) for e in range(NCORES) for c in res.results[e]["nf_out"]]
    out = np.empty((N, D), np.float32)
    if max(counts) > 768:
        # capacity overflow (pathologically imbalanced routing):
        # fall back to the dense variant, which is correct for any routing
        key, in_maps = _prep_in_maps(*args, sparse=False)
        nc = _get_nc_dense(key)
        res = run_bass_kernel_spmd(
            nc, in_maps, core_ids=list(range(NCORES)), trace=_trace
        )
        for e in range(NCORES):
            out[e * NT : (e + 1) * NT] = res.results[e]["y_out"]
    else:
        for e in range(NCORES):
            yo = res.results[e]["y_out"]
            for q in range(4):
                out[(q * 8 + e) * 256 : (q * 8 + e + 1) * 256] = yo[
                    q * 256 : (q + 1) * 256
                ]
    out = out.reshape(np.asarray(x).shape)
    if _trace:
        return out, res
    return out



# revision 9
# speedup vs baseline: 1.5415x; 1.5415x over previous
"""MoE feed-forward (8 experts, top-2, 2 shared experts) on 8 TRN2 NeuronCores.

v2 strategy (expert-parallel, all-to-all combine):
  - Tokens are host-permuted to owner-major order (owner o holds global
    256-blocks b with b%8==o; owner-local ascending == global ascending).
  - Each core routes its own 1024 tokens in fp32 (exact top-2), builds the
    gated comb matrix [E, NT], and AllToAll's it (32 KB) so core e gets its
    expert's comb row for every owner.
  - Core e compacts each owner region (cap 320 slots) with sparse_gather,
    gathers the x rows (bf16), runs the expert FFN in 5 chunks of 512
    slots, and writes UNGATED bf16 outputs contiguously into a payload
    buffer [8*320, D] ordered by owner. One AllToAll (5.2 MB) hands each
    owner its tokens' expert rows.
  - The owner knows (locally, from its own router) which two experts each
    token picked and the token's rank inside each region (prefix-sum via a
    triangular matmul), so the combine is two indirect gathers + gating +
    shared-expert add. No ReduceScatter, no zero-fill, no scatter.
  - Shared experts run data-parallel on the own-token slice: pass 0 hides
    comb-A2A + compaction + gathers; pass 1 hides the payload A2A and the
    combine gathers. w1/w2 stay SBUF-resident; sw2 shares the same SBUF
    tags as w2 (sequenced: sw2(s0) -> w2 -> sw2(s1)).
  - x->xT for the expert stage-1 uses the XBAR dma transpose (verified:
    [128,1024] -> [128,8,128] gives out[:,d,:] = x^T block d).
"""

import sys

if "/opt/trn_rl_repo" not in sys.path:
    sys.path.insert(0, "/opt/trn_rl_repo")

import numpy as np
import ml_dtypes

import concourse.bass as bass
import concourse.tile as tile
import concourse.mybir as mybir
from concourse import bacc
from concourse.bass_utils import run_bass_kernel_spmd

F32 = mybir.dt.float32
F32R = mybir.dt.float32r
BF16 = mybir.dt.bfloat16
I32 = mybir.dt.int32
U32 = mybir.dt.uint32
AX = mybir.AxisListType
ALU = mybir.AluOpType
ACTF = mybir.ActivationFunctionType

N, D, HE, E, S = 8192, 1024, 2048, 8, 2
NCORES = 8
NT = N // NCORES      # 1024 tokens per core
ND = D // 128         # 8
NH = HE // 128        # 16
RG = [list(range(NCORES))]

CAPR = 320            # slots per (expert, owner) region (mean load 256)
TOT = E * CAPR        # 2560 slots per expert core
NCH = TOT // 512      # 5 chunks of 512 slots
NIDX = TOT // 128     # 20 gather-index columns

_NC_CACHE = {}


def _build_v2(has_rb, has_b2, has_sb2, debug=False):
    from concourse import library_config

    nc = bacc.Bacc(None, target_bir_lowering=False)
    if debug:
        dbg_comb_p = nc.declare_dram_parameter("dbg_comb", [E, NT], F32, isOutput=True)
        dbg_a2a_p = nc.declare_dram_parameter("dbg_a2a", [E, NT], F32, isOutput=True)
        dbg_ix_p = nc.declare_dram_parameter("dbg_ix", [128, NIDX], F32, isOutput=True)
        dbg_i12_p = nc.declare_dram_parameter("dbg_i12", [128, 16], I32, isOutput=True)
        dbg_pay_p = nc.declare_dram_parameter("dbg_pay", [TOT, D], BF16, isOutput=True)
        dbg_payo_p = nc.declare_dram_parameter("dbg_payo", [TOT, D], BF16, isOutput=True)
        dbg_sh_p = nc.declare_dram_parameter("dbg_sh", [NT, D], BF16, isOutput=True)
        dbg_xgt_p = nc.declare_dram_parameter("dbg_xgt", [128, 4 * ND * 128], BF16, isOutput=True)

    xtm_p = nc.declare_dram_parameter("xtm", [N, D], BF16, isOutput=False)
    xslf_p = nc.declare_dram_parameter("xslf", [D, NT], F32, isOutput=False)
    xslb_p = nc.declare_dram_parameter("xslb", [D, NT], BF16, isOutput=False)
    w1_p = nc.declare_dram_parameter("w1", [D, HE], BF16, isOutput=False)
    w2_p = nc.declare_dram_parameter("w2", [HE, D], BF16, isOutput=False)
    rw_p = nc.declare_dram_parameter("rw", [128, ND, E], F32, isOutput=False)
    sw1_p = nc.declare_dram_parameter("sw1", [S * D, HE], BF16, isOutput=False)
    sw2_p = nc.declare_dram_parameter("sw2", [S * HE, D], BF16, isOutput=False)
    b1v_p = nc.declare_dram_parameter("b1v", [128, NH], F32, isOutput=False)
    sb1v_p = nc.declare_dram_parameter("sb1v", [128, S * NH], F32, isOutput=False)
    id_p = nc.declare_dram_parameter("ident", [128, 128], F32, isOutput=False)
    lx_p = nc.declare_dram_parameter("lx", [128, 128], F32, isOutput=False)
    io16_p = nc.declare_dram_parameter("io16", [16, NT // 16], F32, isOutput=False)
    eoffs_p = nc.declare_dram_parameter("eoffs", [128, E], F32, isOutput=False)
    ones1_p = nc.declare_dram_parameter("ones1", [1, 128], F32, isOutput=False)
    onec_p = nc.declare_dram_parameter("onec", [128, 1], F32, isOutput=False)
    if has_rb:
        rb_p = nc.declare_dram_parameter("rbr", [128, E], F32, isOutput=False)
    if has_b2:
        b2_p = nc.declare_dram_parameter("b2r", [128, D], F32, isOutput=False)
    if has_sb2:
        sb2_p = nc.declare_dram_parameter("sb2r", [128, D], F32, isOutput=False)
    yo_p = nc.declare_dram_parameter("y_out", [NT, D], F32, isOutput=True)
    nf_p = nc.declare_dram_parameter("nf_out", [E], U32, isOutput=True)

    from contextlib import ExitStack

    with tile.TileContext(nc) as tc, ExitStack() as ctx:
        ep = ctx.enter_context
        dram = ep(tc.tile_pool(name="dram", bufs=1, space="DRAM"))
        cpool = ep(tc.tile_pool(name="cpool", bufs=1))
        xslbp = ep(tc.tile_pool(name="xslbp", bufs=1))
        xfp = ep(tc.tile_pool(name="xfp", bufs=3))
        w1p = ep(tc.tile_pool(name="w1p", bufs=1))
        w2p = ep(tc.tile_pool(name="w2p", bufs=1))
        swtp = ep(tc.tile_pool(name="swtp", bufs=9))
        selp = ep(tc.tile_pool(name="selp", bufs=1))
        rp = ep(tc.tile_pool(name="rp", bufs=4))
        cmp_ = ep(tc.tile_pool(name="cmp", bufs=2))
        cpp = ep(tc.tile_pool(name="cpp", bufs=1))
        xgp = ep(tc.tile_pool(name="xgp", bufs=3))
        xgtp = ep(tc.tile_pool(name="xgtp", bufs=2))
        htp = ep(tc.tile_pool(name="htp", bufs=1))
        ysbp = ep(tc.tile_pool(name="ysbp", bufs=3))
        payp = ep(tc.tile_pool(name="payp", bufs=3))
        ygp = ep(tc.tile_pool(name="ygp", bufs=6))
        finp = ep(tc.tile_pool(name="finp", bufs=2))
        ps1 = ep(tc.tile_pool(name="ps1", bufs=3, space="PSUM"))
        ps2 = ep(tc.tile_pool(name="ps2", bufs=4, space="PSUM"))

        nc.gpsimd.load_library(library_config.sparse_gather)

        a2a_in = dram.tile([E, NT], F32, name="a2a_in")
        a2a_out = dram.tile([E, NT], F32, name="a2a_out")
        pay_in = dram.tile([TOT, D], BF16, name="pay_in")
        pay_out = dram.tile([TOT, D], BF16, name="pay_out")
        idx_d = dram.tile([TOT, 1], F32, name="idx_d")
        sh_y0 = dram.tile([NT, D], BF16, name="sh_y0")
        sh_y1 = dram.tile([NT, D], BF16, name="sh_y1")

        # ---------------- constants -------------------------------------
        ident = cpool.tile([128, 128], F32, name="ident")
        nc.sync.dma_start(ident[:], id_p[:])
        lx = cpool.tile([128, 128], F32, name="lx")
        nc.sync.dma_start(lx[:], lx_p[:])
        rw_sb = cpool.tile([128, ND, E], F32, name="rw_sb")
        nc.sync.dma_start(rw_sb[:], rw_p[:])
        b1v = cpool.tile([128, NH], F32, name="b1v")
        nc.sync.dma_start(b1v[:], b1v_p[:])
        sb1v = cpool.tile([128, S * NH], F32, name="sb1v")
        nc.sync.dma_start(sb1v[:], sb1v_p[:])
        io16 = cpool.tile([16, NT // 16], F32, name="io16")
        nc.sync.dma_start(io16[:], io16_p[:])
        eoffs = cpool.tile([128, E], F32, name="eoffs")
        nc.sync.dma_start(eoffs[:], eoffs_p[:])
        ones1 = cpool.tile([1, 128], F32, name="ones1")
        nc.sync.dma_start(ones1[:], ones1_p[:])
        onec = cpool.tile([128, 1], F32, name="onec")
        nc.sync.dma_start(onec[:], onec_p[:])
        if has_rb:
            rbr = cpool.tile([128, E], F32, name="rbr")
            nc.sync.dma_start(rbr[:], rb_p[:])
        if has_b2:
            b2r = cpool.tile([128, D], F32, name="b2r")
            nc.sync.dma_start(b2r[:], b2_p[:])
        if has_sb2:
            sb2r = cpool.tile([128, D], F32, name="sb2r")
            nc.sync.dma_start(sb2r[:], sb2_p[:])

        # ---------------- router: gatesT = rw.T @ x  (fp32 exact) --------
        pg = [
            ps1.tile([E, 512], F32, tag="hpsum", name=f"pg{half}")
            for half in range(2)
        ]
        for d in range(ND):
            xft = xfp.tile([128, NT], F32, tag="xf", name=f"xf{d}")
            nc.sync.dma_start(xft[:], xslf_p[d * 128 : (d + 1) * 128, :])
            for half in range(2):
                nc.tensor.matmul(
                    pg[half][:],
                    rw_sb[:, d, :],
                    xft[:, half * 512 : (half + 1) * 512],
                    start=(d == 0),
                    stop=(d == ND - 1),
                )
        combT_g = cpp.tile([E, NT], F32, name="combT_g")
        for half in range(2):
            nc.vector.tensor_copy(
                combT_g[:, half * 512 : (half + 1) * 512], pg[half][:]
            )

        # own-token bf16 slice for the shared experts (early DMA)
        xslb = []
        for d in range(ND):
            t = xslbp.tile([128, NT], BF16, tag=f"xb{d}", name=f"xb{d}")
            nc.sync.dma_start(t[:], xslb_p[d * 128 : (d + 1) * 128, :])
            xslb.append(t)

        # ---------------- top-2 selection (token-major) ------------------
        combA2A = cpp.tile([E, NT], F32, name="combA2A")
        mjs, m1s, m2s, p1s, p2s = [], [], [], [], []
        for j in range(NT // 128):
            gtp = ps1.tile([128, E], F32, tag="hpsum", name=f"gtp{j}")
            nc.tensor.transpose(
                gtp[:], combT_g[:, j * 128 : (j + 1) * 128], ident[0:E, 0:E]
            )
            gates = rp.tile([128, E], F32, tag="gates", name=f"gates{j}")
            if has_rb:
                nc.vector.tensor_tensor(gates[:], gtp[:], rbr[:], op=ALU.add)
            else:
                nc.vector.tensor_copy(gates[:], gtp[:])
            m1 = rp.tile([128, 1], F32, tag="m1", name=f"m1_{j}")
            nc.vector.tensor_reduce(m1[:], gates[:], axis=AX.X, op=ALU.max)
            mask1 = selp.tile([128, E], F32, tag=f"mask1_{j}", name=f"mask1_{j}")
            nc.vector.tensor_scalar(mask1[:], gates[:], m1[:], None, op0=ALU.is_equal)
            negm = rp.tile([128, E], F32, tag="negm", name=f"negm{j}")
            nc.vector.tensor_scalar(negm[:], mask1[:], -1e30, None, op0=ALU.mult)
            gm = rp.tile([128, E], F32, tag="gm", name=f"gm{j}")
            nc.vector.tensor_tensor(gm[:], gates[:], negm[:], op=ALU.add)
            m2 = rp.tile([128, 1], F32, tag="m2", name=f"m2_{j}")
            nc.vector.tensor_reduce(m2[:], gm[:], axis=AX.X, op=ALU.max)
            mask2 = selp.tile([128, E], F32, tag=f"mask2_{j}", name=f"mask2_{j}")
            nc.vector.tensor_scalar(mask2[:], gm[:], m2[:], None, op0=ALU.is_equal)
            dl = rp.tile([128, 1], F32, tag="dl", name=f"dl{j}")
            nc.vector.tensor_tensor(dl[:], m2[:], m1[:], op=ALU.subtract)
            e2 = rp.tile([128, 1], F32, tag="e2", name=f"e2_{j}")
            nc.scalar.activation(e2[:], dl[:], ACTF.Exp)
            den = rp.tile([128, 1], F32, tag="den", name=f"den{j}")
            nc.vector.tensor_scalar_add(den[:], e2[:], 1.0)
            p1 = selp.tile([128, 1], F32, tag=f"p1_{j}", name=f"p1_{j}")
            nc.vector.reciprocal(p1[:], den[:])
            p2 = selp.tile([128, 1], F32, tag=f"p2_{j}", name=f"p2_{j}")
            nc.vector.tensor_tensor(p2[:], e2[:], p1[:], op=ALU.mult)
            t1 = rp.tile([128, E], F32, tag="t1", name=f"t1_{j}")
            nc.vector.tensor_scalar(t1[:], mask1[:], p1[:], None, op0=ALU.mult)
            t2 = rp.tile([128, E], F32, tag="t2", name=f"t2_{j}")
            nc.vector.tensor_scalar(t2[:], mask2[:], p2[:], None, op0=ALU.mult)
            cj = rp.tile([128, E], F32, tag="cj", name=f"cj{j}")
            nc.vector.tensor_tensor(cj[:], t1[:], t2[:], op=ALU.add)
            # mj from cj (not mask1+mask2) so owner ranks agree bit-exactly
            # with the expert-side compaction of the A2A'd comb values
            mj = selp.tile([128, E], F32, tag=f"mj_{j}", name=f"mj_{j}")
            nc.vector.tensor_scalar(mj[:], cj[:], 0.0, None, op0=ALU.not_equal)
            pt = ps1.tile([E, 128], F32, tag="hpsum", name=f"pt{j}")
            nc.tensor.transpose(pt[:], cj[:], ident[:])
            nc.vector.tensor_copy(combA2A[:, j * 128 : (j + 1) * 128], pt[:])
            mjs.append(mj)
            m1s.append(mask1)
            m2s.append(mask2)
            p1s.append(p1)
            p2s.append(p2)
        nc.sync.dma_start(a2a_in[:], combA2A[:])
        nc.gpsimd.collective_compute(
            "AllToAll",
            ALU.bypass,
            replica_groups=RG,
            ins=[a2a_in.opt()],
            outs=[a2a_out.opt()],
        )

        # ---------------- owner-side ranks -> gather indices i1/i2 -------
        runv = cpp.tile([1, E], F32, name="runv")
        nc.vector.memset(runv[:], 0.0)
        i1s, i2s = [], []
        for j in range(NT // 128):
            pr = ps1.tile([128, E], F32, tag="hpsum", name=f"pr{j}")
            nc.tensor.matmul(pr[:], lx[:], mjs[j][:], start=True, stop=False)
            nc.tensor.matmul(pr[:], ones1[:], runv[:], start=False, stop=True)
            flat = rp.tile([128, E], F32, tag="flat", name=f"flat{j}")
            nc.vector.tensor_tensor(flat[:], pr[:], eoffs[:], op=ALU.add)
            tb = ps1.tile([1, E], F32, tag="hpsum", name=f"tb{j}")
            nc.tensor.matmul(tb[:], onec[:], mjs[j][:], start=True, stop=True)
            nc.vector.tensor_tensor(runv[:], runv[:], tb[:], op=ALU.add)
            f1 = rp.tile([128, E], F32, tag="f1", name=f"f1_{j}")
            nc.vector.tensor_tensor(f1[:], flat[:], m1s[j][:], op=ALU.mult)
            i1f = rp.tile([128, 1], F32, tag="i1f", name=f"i1f{j}")
            nc.vector.tensor_reduce(i1f[:], f1[:], axis=AX.X, op=ALU.max)
            i1 = selp.tile([128, 1], I32, tag=f"i1_{j}", name=f"i1_{j}")
            nc.vector.tensor_copy(i1[:], i1f[:])
            f2 = rp.tile([128, E], F32, tag="f2", name=f"f2_{j}")
            nc.vector.tensor_tensor(f2[:], flat[:], m2s[j][:], op=ALU.mult)
            i2f = rp.tile([128, 1], F32, tag="i2f", name=f"i2f{j}")
            nc.vector.tensor_reduce(i2f[:], f2[:], axis=AX.X, op=ALU.max)
            i2 = selp.tile([128, 1], I32, tag=f"i2_{j}", name=f"i2_{j}")
            nc.vector.tensor_copy(i2[:], i2f[:])
            i1s.append(i1)
            i2s.append(i2)

        # ---------------- expert-side compaction (per owner region) ------
        nf_all = cpp.tile([1, E], U32, name="nf_all")
        for r in range(E):
            c16 = cmp_.tile([16, NT // 16], F32, tag="c16", name=f"c16_{r}")
            nc.sync.dma_start(c16[:], a2a_out[r, :].rearrange("(f p) -> p f", p=16))
            msk = cmp_.tile([16, NT // 16], F32, tag="msk", name=f"msk{r}")
            nc.vector.tensor_scalar(msk[:], c16[:], 0.0, None, op0=ALU.not_equal)
            mm1 = cmp_.tile([16, NT // 16], F32, tag="mm1", name=f"mm1{r}")
            nc.vector.tensor_scalar(mm1[:], msk[:], 1.0, None, op0=ALU.subtract)
            av = cmp_.tile([16, NT // 16], F32, tag="av", name=f"av{r}")
            nc.vector.tensor_tensor(av[:], io16[:], msk[:], op=ALU.mult)
            nc.vector.tensor_tensor(av[:], av[:], mm1[:], op=ALU.add)
            idxc = cmp_.tile([16, CAPR // 16], F32, tag="idxc", name=f"idxc{r}")
            nc.vector.memset(idxc[:], 0.0)
            nfr = cmp_.tile([1, 1], U32, tag="nfr", name=f"nfr{r}")
            nc.gpsimd.sparse_gather(idxc[:], av[:], num_found=nfr[:])
            nc.vector.tensor_copy(nf_all[:, r : r + 1], nfr[:])
            # sparse_gather leaves garbage past num_found: zero the tail
            # (slot >= nf) so pad slots gather a valid row instead
            nff = cmp_.tile([1, 1], F32, tag="nff", name=f"nff{r}")
            nc.vector.tensor_copy(nff[:], nfr[:])
            nfb_ps = ps1.tile([16, 1], F32, tag="hpsum", name=f"nfb{r}")
            nc.tensor.matmul(nfb_ps[:], ones1[:, 0:16], nff[:], start=True, stop=True)
            nfb = cmp_.tile([16, 1], F32, tag="nfb", name=f"nfbs{r}")
            nc.vector.tensor_copy(nfb[:], nfb_ps[:])
            mt = cmp_.tile([16, CAPR // 16], F32, tag="mt", name=f"mt{r}")
            nc.vector.tensor_scalar(
                mt[:], io16[:, 0 : CAPR // 16], nfb[:], None, op0=ALU.is_ge
            )
            imt = cmp_.tile([16, CAPR // 16], F32, tag="imt", name=f"imt{r}")
            nc.vector.tensor_scalar(imt[:], mt[:], -1.0, None, op0=ALU.mult)
            nc.vector.tensor_scalar_add(imt[:], imt[:], 1.0)
            nc.vector.tensor_tensor(idxc[:], idxc[:], imt[:], op=ALU.mult)
            nc.vector.tensor_scalar(idxc[:], idxc[:], 0.0, None, op0=ALU.max)
            nc.vector.tensor_scalar(idxc[:], idxc[:], float(NT - 1), None, op0=ALU.min)
            idxg = cmp_.tile([16, CAPR // 16], F32, tag="idxg", name=f"idxg{r}")
            nc.vector.tensor_scalar_add(idxg[:], idxc[:], float(r * NT))
            nc.sync.dma_start(
                idx_d[r * CAPR : (r + 1) * CAPR, :].rearrange(
                    "(f p) one -> p (f one)", p=16
                ),
                idxg[:],
            )
        ix = cpp.tile([128, NIDX], F32, name="ixf")
        nc.sync.dma_start(ix[:], idx_d[:].rearrange("(c p) one -> p (c one)", p=128))
        ixi = cpp.tile([128, NIDX], I32, name="ixi")
        nc.vector.tensor_copy(ixi[:], ix[:])
        nc.sync.dma_start(nf_p[:], nf_all[:])
        if debug:
            nc.sync.dma_start(dbg_comb_p[:], combA2A[:])
            nc.sync.dma_start(dbg_a2a_p[:], a2a_out[:])
            nc.sync.dma_start(dbg_ix_p[:], ix[:])
            for j in range(NT // 128):
                nc.sync.dma_start(dbg_i12_p[:, j : j + 1], i1s[j][:])
                nc.sync.dma_start(dbg_i12_p[:, 8 + j : 8 + j + 1], i2s[j][:])

        # ---------------- shared experts ---------------------------------
        hts = [None] * NH
        w2sb = [None] * NH

        def shared_pass(s, ydst):
            for t in range(2):
                for hq in range(4):
                    swts = []
                    for d in range(ND):
                        w = swtp.tile(
                            [128, 512], BF16, tag="swt", name=f"swt{s}_{t}_{hq}_{d}"
                        )
                        nc.scalar.dma_start(
                            w[:],
                            sw1_p[
                                s * D + d * 128 : s * D + (d + 1) * 128,
                                hq * 512 : (hq + 1) * 512,
                            ],
                        )
                        swts.append(w)
                    for h4 in range(4):
                        h = hq * 4 + h4
                        ph = ps1.tile(
                            [128, 512], F32, tag="hpsum", name=f"shp{s}_{t}_{h}"
                        )
                        for d in range(ND):
                            nc.tensor.matmul(
                                ph[:],
                                swts[d][:, h4 * 128 : (h4 + 1) * 128],
                                xslb[d][:, t * 512 : (t + 1) * 512],
                                start=(d == 0),
                                stop=(d == ND - 1),
                            )
                        ht = htp.tile(
                            [128, 512], BF16, tag=f"hT{h}", name=f"shT{s}_{t}_{h}"
                        )
                        nc.scalar.activation(
                            ht[:],
                            ph[:],
                            ACTF.Gelu,
                            bias=sb1v[:, s * NH + h : s * NH + h + 1],
                        )
                        hts[h] = ht
                for k in range(4):
                    ysb = ysbp.tile([128, D], BF16, tag="ysb", name=f"shy{s}_{t}_{k}")
                    for v in range(2):
                        py = ps2.tile(
                            [128, 512], F32, tag="ypsum", name=f"spy{s}_{t}_{k}_{v}"
                        )
                        for h in range(NH):
                            nc.tensor.matmul(
                                py[:],
                                hts[h][:, k * 128 : (k + 1) * 128],
                                w2sb[h][:, v * 512 : (v + 1) * 512],
                                start=(h == 0),
                                stop=(h == NH - 1),
                            )
                        nc.vector.tensor_copy(ysb[:, v * 512 : (v + 1) * 512], py[:])
                    g = t * 4 + k
                    nc.sync.dma_start(ydst[g * 128 : (g + 1) * 128, :], ysb[:])

        # shared pass 0: the w2 tags hold sw2 (expert 0) rows
        for h in range(NH):
            w = w2p.tile([128, D], BF16, tag=f"w2_{h}", name=f"sw2a_{h}")
            nc.scalar.dma_start(w[:], sw2_p[h * 128 : (h + 1) * 128, :])
            w2sb[h] = w
        shared_pass(0, sh_y0)

        # ---------------- expert FFN (5 chunks of 512 slots) -------------
        w1sb = []
        for d in range(ND):
            w = w1p.tile([128, HE], BF16, tag=f"w1_{d}", name=f"w1_{d}")
            nc.scalar.dma_start(w[:], w1_p[d * 128 : (d + 1) * 128, :])
            w1sb.append(w)
        for h in range(NH):
            w = w2p.tile([128, D], BF16, tag=f"w2_{h}", name=f"w2e_{h}")
            nc.scalar.dma_start(w[:], w2_p[h * 128 : (h + 1) * 128, :])
            w2sb[h] = w

        def emit_gathers(c):
            tiles = []
            for i in range(4):
                xg = xgp.tile([128, D], BF16, tag="xg", name=f"xg{c}_{i}")
                nc.gpsimd.indirect_dma_start(
                    out=xg[:],
                    out_offset=None,
                    in_=xtm_p[:],
                    in_offset=bass.IndirectOffsetOnAxis(
                        ap=ixi[:, 4 * c + i : 4 * c + i + 1], axis=0
                    ),
                    bounds_check=N - 1,
                    oob_is_err=False,
                )
                tiles.append(xg)
            return tiles

        xg_next = emit_gathers(0)
        for c in range(NCH):
            xg_cur = xg_next
            xgt = xgtp.tile([128, 4, ND, 128], BF16, tag="xgt", name=f"xgt{c}")
            for i in range(4):
                nc.sync.dma_start_transpose(xgt[:, i, :, :], xg_cur[i][:])
            if debug and c == 0:
                nc.sync.dma_start(
                    dbg_xgt_p[:], xgt[:].rearrange("p a b c -> p (a b c)")
                )
            if c + 1 < NCH:
                xg_next = emit_gathers(c + 1)
            for h in range(NH):
                ph = ps1.tile([128, 512], F32, tag="hpsum", name=f"eph{c}_{h}")
                for d in range(ND):
                    nc.tensor.matmul(
                        ph[:],
                        w1sb[d][:, h * 128 : (h + 1) * 128],
                        xgt[:, :, d, :],
                        start=(d == 0),
                        stop=(d == ND - 1),
                    )
                ht = htp.tile([128, 512], BF16, tag=f"hT{h}", name=f"ehT{c}_{h}")
                nc.scalar.activation(ht[:], ph[:], ACTF.Gelu, bias=b1v[:, h : h + 1])
                hts[h] = ht
            for g in range(4):
                pay = payp.tile([128, D], BF16, tag="pay", name=f"pay{c}_{g}")
                for v in range(2):
                    py = ps2.tile([128, 512], F32, tag="ypsum", name=f"epy{c}_{g}_{v}")
                    for h in range(NH):
                        nc.tensor.matmul(
                            py[:],
                            hts[h][:, g * 128 : (g + 1) * 128],
                            w2sb[h][:, v * 512 : (v + 1) * 512],
                            start=(h == 0),
                            stop=(h == NH - 1),
                        )
                    if has_b2:
                        nc.vector.tensor_tensor(
                            py[:], py[:], b2r[:, v * 512 : (v + 1) * 512], op=ALU.add
                        )
                    nc.vector.tensor_copy(pay[:, v * 512 : (v + 1) * 512], py[:])
                row = (c * 4 + g) * 128
                nc.sync.dma_start(pay_in[row : row + 128, :], pay[:])

        nc.gpsimd.collective_compute(
            "AllToAll",
            ALU.bypass,
            replica_groups=RG,
            ins=[pay_in.opt()],
            outs=[pay_out.opt()],
        )

        # shared pass 1: reload the w2 tags with sw2 (expert 1) rows
        for h in range(NH):
            w = w2p.tile([128, D], BF16, tag=f"w2_{h}", name=f"sw2b_{h}")
            nc.scalar.dma_start(w[:], sw2_p[HE + h * 128 : HE + (h + 1) * 128, :])
            w2sb[h] = w
        shared_pass(1, sh_y1)
        if debug:
            nc.sync.dma_start(dbg_pay_p[:], pay_in[:])
            nc.sync.dma_start(dbg_payo_p[:], pay_out[:])
            nc.sync.dma_start(dbg_sh_p[:], sh_y0[:])

        # ---------------- owner-side combine ------------------------------
        for j in range(NT // 128):
            yg1 = ygp.tile([128, D], BF16, tag="yg", name=f"yg1_{j}")
            nc.gpsimd.indirect_dma_start(
                out=yg1[:],
                out_offset=None,
                in_=pay_out[:],
                in_offset=bass.IndirectOffsetOnAxis(ap=i1s[j][:], axis=0),
                bounds_check=TOT - 1,
                oob_is_err=False,
            )
            yg2 = ygp.tile([128, D], BF16, tag="yg", name=f"yg2_{j}")
            nc.gpsimd.indirect_dma_start(
                out=yg2[:],
                out_offset=None,
                in_=pay_out[:],
                in_offset=bass.IndirectOffsetOnAxis(ap=i2s[j][:], axis=0),
                bounds_check=TOT - 1,
                oob_is_err=False,
            )
            sh0 = ygp.tile([128, D], BF16, tag="yg", name=f"sh0_{j}")
            nc.sync.dma_start(sh0[:], sh_y0[j * 128 : (j + 1) * 128, :])
            sh1 = ygp.tile([128, D], BF16, tag="yg", name=f"sh1_{j}")
            nc.sync.dma_start(sh1[:], sh_y1[j * 128 : (j + 1) * 128, :])
            fin = finp.tile([128, D], F32, tag="fin", name=f"fin{j}")
            nc.vector.tensor_scalar(fin[:], yg1[:], p1s[j][:], None, op0=ALU.mult)
            f2t = finp.tile([128, D], F32, tag="fin", name=f"f2t{j}")
            nc.vector.tensor_scalar(f2t[:], yg2[:], p2s[j][:], None, op0=ALU.mult)
            nc.vector.tensor_tensor(fin[:], fin[:], f2t[:], op=ALU.add)
            nc.vector.tensor_tensor(fin[:], fin[:], sh0[:], op=ALU.add)
            nc.vector.tensor_tensor(fin[:], fin[:], sh1[:], op=ALU.add)
            if has_sb2:
                nc.vector.tensor_tensor(fin[:], fin[:], sb2r[:], op=ALU.add)
            nc.sync.dma_start(yo_p[j * 128 : (j + 1) * 128, :], fin[:])

    nc.compile()
    return nc


def _build(has_rb, has_b2, has_sb2, debug=False):
    """Dense expert-parallel fallback (correct for any routing)."""
    nc = bacc.Bacc(None, target_bir_lowering=False)

    TBC = 1024
    NTBC = N // TBC

    xt_p = nc.declare_dram_parameter("xt", [D, N], F32R, isOutput=False)
    xsl_p = nc.declare_dram_parameter("xsl", [D, NT], F32R, isOutput=False)
    w1_p = nc.declare_dram_parameter("w1", [D, HE], F32R, isOutput=False)
    w2_p = nc.declare_dram_parameter("w2", [HE, D], BF16, isOutput=False)
    rw_p = nc.declare_dram_parameter("rw", [128, ND, E], F32R, isOutput=False)
    sw1_p = nc.declare_dram_parameter("sw1", [S * D, HE], F32R, isOutput=False)
    sw2_p = nc.declare_dram_parameter("sw2", [S * HE, D], BF16, isOutput=False)
    b1v_p = nc.declare_dram_parameter("b1v", [128, NH], F32, isOutput=False)
    sb1v_p = nc.declare_dram_parameter("sb1v", [128, S * NH], F32, isOutput=False)
    id_p = nc.declare_dram_parameter("ident", [128, 128], F32, isOutput=False)
    if has_rb:
        rb_p = nc.declare_dram_parameter("rbr", [128, E], F32, isOutput=False)
    if has_b2:
        b2_p = nc.declare_dram_parameter("b2r", [128, D], F32, isOutput=False)
    if has_sb2:
        sb2_p = nc.declare_dram_parameter("sb2r", [128, D], F32, isOutput=False)
    yo_p = nc.declare_dram_parameter("y_out", [NT, D], F32, isOutput=True)

    from contextlib import ExitStack

    with tile.TileContext(nc) as tc, ExitStack() as ctx:
        ep = ctx.enter_context
        dram = ep(tc.tile_pool(name="dram", bufs=1, space="DRAM"))
        cpool = ep(tc.tile_pool(name="cpool", bufs=1))
        xslp = ep(tc.tile_pool(name="xslp", bufs=1))
        xtp = ep(tc.tile_pool(name="xtp", bufs=1))
        htp = ep(tc.tile_pool(name="htp", bufs=1))
        w2rp = ep(tc.tile_pool(name="w2rp", bufs=1))
        wst = ep(tc.tile_pool(name="wst", bufs=6))
        sw2st = ep(tc.tile_pool(name="sw2st", bufs=3))
        ysbp = ep(tc.tile_pool(name="ysbp", bufs=2))
        finp = ep(tc.tile_pool(name="finp", bufs=2))
        rp = ep(tc.tile_pool(name="rp", bufs=2))
        ps1 = ep(tc.tile_pool(name="ps1", bufs=2, space="PSUM"))
        ps2 = ep(tc.tile_pool(name="ps2", bufs=2, space="PSUM"))

        moe_y = dram.tile([N, D], F32, name="moe_y")
        rs_out = dram.tile([NT, D], F32, name="rs_out")
        a2a_in = dram.tile([E, NT], F32, name="a2a_in")
        a2a_out = dram.tile([E, NT], F32, name="a2a_out")

        ident = cpool.tile([128, 128], F32, name="ident")
        nc.sync.dma_start(ident[:], id_p[:])
        b1v = cpool.tile([128, NH], F32, name="b1v")
        nc.sync.dma_start(b1v[:], b1v_p[:])
        sb1v = cpool.tile([128, S * NH], F32, name="sb1v")
        nc.sync.dma_start(sb1v[:], sb1v_p[:])
        rw_sb = cpool.tile([128, ND, E], F32, name="rw_sb")
        nc.sync.dma_start(rw_sb[:], rw_p[:].bitcast(F32))
        if has_rb:
            rbr = cpool.tile([128, E], F32, name="rbr")
            nc.sync.dma_start(rbr[:], rb_p[:])
        if has_b2:
            b2r = cpool.tile([128, D], F32, name="b2r")
            nc.sync.dma_start(b2r[:], b2_p[:])
        if has_sb2:
            sb2r = cpool.tile([128, D], F32, name="sb2r")
            nc.sync.dma_start(sb2r[:], sb2_p[:])

        xsl = []
        for d in range(ND):
            t = xslp.tile([128, NT], F32R, tag=f"xsl{d}", name=f"xsl{d}")
            nc.sync.dma_start(t[:], xsl_p[d * 128 : (d + 1) * 128, :])
            xsl.append(t)

        combT = cpool.tile([E, NT], F32, name="combT")
        for j in range(NT // 128):
            pg = ps1.tile([128, E], F32, tag="hpsum", name=f"pg{j}")
            for d in range(ND):
                xr = rp.tile([128, 128], F32, tag="xr", name=f"xr{j}_{d}", bufs=4)
                nc.sync.dma_start(
                    xr[:],
                    xsl_p[d * 128 : (d + 1) * 128, j * 128 : (j + 1) * 128].bitcast(
                        F32
                    ),
                )
                nc.tensor.matmul(
                    pg[:], xr[:], rw_sb[:, d, :], start=(d == 0), stop=(d == ND - 1)
                )
            gates = rp.tile([128, E], F32, tag="gates", name=f"gates{j}")
            if has_rb:
                nc.vector.tensor_tensor(gates[:], pg[:], rbr[:], op=ALU.add)
            else:
                nc.vector.tensor_copy(gates[:], pg[:])
            m1 = rp.tile([128, 1], F32, tag="m1", name=f"m1_{j}")
            nc.vector.tensor_reduce(m1[:], gates[:], axis=AX.X, op=ALU.max)
            mask1 = rp.tile([128, E], F32, tag="mask1", name=f"mask1_{j}")
            nc.vector.tensor_scalar(mask1[:], gates[:], m1[:], None, op0=ALU.is_equal)
            negm = rp.tile([128, E], F32, tag="negm", name=f"negm{j}")
            nc.vector.tensor_scalar(negm[:], mask1[:], -1e30, None, op0=ALU.mult)
            gm = rp.tile([128, E], F32, tag="gm", name=f"gm{j}")
            nc.vector.tensor_tensor(gm[:], gates[:], negm[:], op=ALU.add)
            m2 = rp.tile([128, 1], F32, tag="m2", name=f"m2_{j}")
            nc.vector.tensor_reduce(m2[:], gm[:], axis=AX.X, op=ALU.max)
            mask2 = rp.tile([128, E], F32, tag="mask2", name=f"mask2_{j}")
            nc.vector.tensor_scalar(mask2[:], gm[:], m2[:], None, op0=ALU.is_equal)
            dl = rp.tile([128, 1], F32, tag="dl", name=f"dl{j}")
            nc.vector.tensor_tensor(dl[:], m2[:], m1[:], op=ALU.subtract)
            e2 = rp.tile([128, 1], F32, tag="e2", name=f"e2_{j}")
            nc.scalar.activation(e2[:], dl[:], ACTF.Exp)
            den = rp.tile([128, 1], F32, tag="den", name=f"den{j}")
            nc.vector.tensor_scalar_add(den[:], e2[:], 1.0)
            p1 = rp.tile([128, 1], F32, tag="p1", name=f"p1_{j}")
            nc.vector.reciprocal(p1[:], den[:])
            p2 = rp.tile([128, 1], F32, tag="p2", name=f"p2_{j}")
            nc.vector.tensor_tensor(p2[:], e2[:], p1[:], op=ALU.mult)
            t1 = rp.tile([128, E], F32, tag="t1", name=f"t1_{j}")
            nc.vector.tensor_scalar(t1[:], mask1[:], p1[:], None, op0=ALU.mult)
            t2 = rp.tile([128, E], F32, tag="t2", name=f"t2_{j}")
            nc.vector.tensor_scalar(t2[:], mask2[:], p2[:], None, op0=ALU.mult)
            cj = rp.tile([128, E], F32, tag="cj", name=f"cj{j}")
            nc.vector.tensor_tensor(cj[:], t1[:], t2[:], op=ALU.add)
            pt = ps1.tile([E, 128], F32, tag="hpsum", name=f"pt{j}")
            nc.tensor.transpose(pt[:], cj[:], ident[:])
            nc.vector.tensor_copy(combT[:, j * 128 : (j + 1) * 128], pt[:])
        nc.sync.dma_start(a2a_in[:], combT[:])
        nc.gpsimd.collective_compute(
            "AllToAll",
            ALU.bypass,
            replica_groups=RG,
            ins=[a2a_in.opt()],
            outs=[a2a_out.opt()],
        )
        comb_tm = cpool.tile([128, N // 128], F32, name="comb_tm")
        nc.sync.dma_start(comb_tm[:], a2a_out[:].rearrange("a (c p) -> p (a c)", p=128))

        w2res = []
        for h in range(NH):
            t = w2rp.tile([128, D], BF16, tag=f"w2r{h}", name=f"w2r{h}")
            nc.sync.dma_start(t[:], w2_p[h * 128 : (h + 1) * 128, :])
            w2res.append(t)

        for tb in range(NTBC):
            xts = []
            for d in range(ND):
                t = xtp.tile([128, TBC], F32R, tag=f"xt{d}", name=f"xt{tb}_{d}")
                nc.sync.dma_start(
                    t[:], xt_p[d * 128 : (d + 1) * 128, tb * TBC : (tb + 1) * TBC]
                )
                xts.append(t)
            hts = []
            for h in range(NH):
                ph = ps1.tile([128, TBC], F32, tag="hpsum", name=f"ph{tb}_{h}")
                for d in range(ND):
                    w1t = wst.tile([128, 128], F32R, tag="w1t", name=f"w1t{tb}_{h}_{d}")
                    nc.sync.dma_start(
                        w1t[:], w1_p[d * 128 : (d + 1) * 128, h * 128 : (h + 1) * 128]
                    )
                    for v in range(TBC // 512):
                        nc.tensor.matmul(
                            ph[:, v * 512 : (v + 1) * 512],
                            w1t[:],
                            xts[d][:, v * 512 : (v + 1) * 512],
                            start=(d == 0),
                            stop=(d == ND - 1),
                        )
                ht = htp.tile([128, TBC], BF16, tag=f"hT{h}", name=f"hT{tb}_{h}")
                nc.scalar.activation(ht[:], ph[:], ACTF.Gelu, bias=b1v[:, h : h + 1])
                hts.append(ht)
            for k in range(TBC // 128):
                g = tb * (TBC // 128) + k
                py = ps2.tile([128, D], F32, tag="ypsum", name=f"py{g}")
                for h in range(NH):
                    for v in range(D // 512):
                        nc.tensor.matmul(
                            py[:, v * 512 : (v + 1) * 512],
                            hts[h][:, k * 128 : (k + 1) * 128],
                            w2res[h][:, v * 512 : (v + 1) * 512],
                            start=(h == 0),
                            stop=(h == NH - 1),
                        )
                if has_b2:
                    nc.vector.tensor_tensor(py[:], py[:], b2r[:], op=ALU.add)
                ysb = ysbp.tile([128, D], F32, tag="ysb", name=f"ysb{g}")
                nc.vector.tensor_scalar(
                    ysb[:], py[:], comb_tm[:, g : g + 1], None, op0=ALU.mult
                )
                nc.sync.dma_start(moe_y[g * 128 : (g + 1) * 128, :], ysb[:])

        nc.gpsimd.collective_compute(
            "ReduceScatter",
            ALU.add,
            replica_groups=RG,
            ins=[moe_y.opt()],
            outs=[rs_out.opt()],
        )

        ysh = []
        for k in range(NT // 128):
            t = xtp.tile([128, D], F32, tag=f"xt{k}", name=f"ysh{k}")
            ysh.append(t)
        for s in range(S):
            shts = []
            for h in range(NH):
                ph = ps1.tile([128, NT], F32, tag="hpsum", name=f"shp{s}_{h}")
                for d in range(ND):
                    swt = wst.tile([128, 128], F32R, tag="w1t", name=f"swt{s}_{h}_{d}")
                    nc.sync.dma_start(
                        swt[:],
                        sw1_p[
                            s * D + d * 128 : s * D + (d + 1) * 128,
                            h * 128 : (h + 1) * 128,
                        ],
                    )
                    for v in range(NT // 512):
                        nc.tensor.matmul(
                            ph[:, v * 512 : (v + 1) * 512],
                            swt[:],
                            xsl[d][:, v * 512 : (v + 1) * 512],
                            start=(d == 0),
                            stop=(d == ND - 1),
                        )
                sht = htp.tile([128, NT], BF16, tag=f"hT{h}", name=f"shT{s}_{h}")
                nc.scalar.activation(
                    sht[:], ph[:], ACTF.Gelu, bias=sb1v[:, s * NH + h : s * NH + h + 1]
                )
                shts.append(sht)
            for kg in range(NT // 256):
                pys = []
                for ki in range(2):
                    k = kg * 2 + ki
                    pys.append(ps2.tile([128, D], F32, tag="ypsum", name=f"spy{s}_{k}"))
                for h in range(NH):
                    sw2t = sw2st.tile([128, D], BF16, tag="sw2t", name=f"sw2t{s}_{kg}_{h}")
                    nc.sync.dma_start(
                        sw2t[:], sw2_p[s * HE + h * 128 : s * HE + (h + 1) * 128, :]
                    )
                    for ki in range(2):
                        k = kg * 2 + ki
                        for v in range(D // 512):
                            nc.tensor.matmul(
                                pys[ki][:, v * 512 : (v + 1) * 512],
                                shts[h][:, k * 128 : (k + 1) * 128],
                                sw2t[:, v * 512 : (v + 1) * 512],
                                start=(h == 0),
                                stop=(h == NH - 1),
                            )
                for ki in range(2):
                    k = kg * 2 + ki
                    if s == 0:
                        nc.vector.tensor_copy(ysh[k][:], pys[ki][:])
                    else:
                        nc.vector.tensor_tensor(ysh[k][:], ysh[k][:], pys[ki][:], op=ALU.add)

        for k in range(NT // 128):
            fin = finp.tile([128, D], F32, tag="fin", name=f"fin{k}")
            nc.sync.dma_start(fin[:], rs_out[k * 128 : (k + 1) * 128, :])
            nc.vector.tensor_tensor(fin[:], fin[:], ysh[k][:], op=ALU.add)
            if has_sb2:
                nc.vector.tensor_tensor(fin[:], fin[:], sb2r[:], op=ALU.add)
            nc.sync.dma_start(yo_p[k * 128 : (k + 1) * 128, :], fin[:])

    nc.compile()
    return nc


def _get_nc_v2(key):
    k2 = ("v2",) + key
    if k2 not in _NC_CACHE:
        _NC_CACHE[k2] = _build_v2(*key)
    return _NC_CACHE[k2]


def _get_nc_dense(key):
    k2 = ("dense",) + key
    if k2 not in _NC_CACHE:
        _NC_CACHE[k2] = _build(*key)
    return _NC_CACHE[k2]


def _flags(router_b, b2, sb2):
    f32 = np.float32
    has_rb = bool(np.any(router_b))
    has_b2 = bool(np.any(b2))
    sb2_eff = np.asarray(sb2, f32).sum(0) / S
    has_sb2 = bool(np.any(sb2_eff))
    return (has_rb, has_b2, has_sb2), sb2_eff


def _prep_v2(x, router_w, router_b, w1, b1, w2, b2, sw1, sb1, sw2, sb2):
    f32 = np.float32
    x2 = np.ascontiguousarray(np.asarray(x, f32).reshape(N, D))
    key, sb2_eff = _flags(router_b, b2, sb2)
    has_rb, has_b2, has_sb2 = key

    # owner-major permutation: xperm[o*NT + q*256 + t] = x2[(q*8+o)*256 + t]
    xperm = np.empty_like(x2)
    for o in range(NCORES):
        for q in range(4):
            xperm[o * NT + q * 256 : o * NT + (q + 1) * 256] = x2[
                (q * 8 + o) * 256 : (q * 8 + o + 1) * 256
            ]
    xperm_bf = xperm.astype(ml_dtypes.bfloat16)

    rw_r = np.ascontiguousarray(
        np.asarray(router_w, f32).reshape(ND, 128, E).transpose(1, 0, 2)
    )
    ident = np.eye(128, dtype=f32)
    lxm = np.triu(np.ones((128, 128), f32), 1)   # lx[c,p]=1 iff c<p
    io16 = (np.arange(NT // 16)[None, :] * 16 + np.arange(16)[:, None]).astype(f32)
    eoffs = np.tile((np.arange(E) * CAPR).astype(f32), (128, 1))
    ones1 = np.ones((1, 128), f32)
    onec = np.ones((128, 1), f32)
    sw1s = np.ascontiguousarray(np.asarray(sw1, f32).reshape(S * D, HE)).astype(
        ml_dtypes.bfloat16
    )
    sw2s = np.ascontiguousarray(
        (np.asarray(sw2, f32) * (1.0 / S)).reshape(S * HE, D)
    ).astype(ml_dtypes.bfloat16)
    sb1v = np.ascontiguousarray(
        np.asarray(sb1, f32).reshape(S, NH, 128).transpose(2, 0, 1).reshape(128, S * NH)
    )

    in_maps = []
    for e in range(NCORES):
        own = xperm[e * NT : (e + 1) * NT]
        ownT = np.ascontiguousarray(own.T)
        m = {
            "xtm": xperm_bf,
            "xslf": ownT,
            "xslb": ownT.astype(ml_dtypes.bfloat16),
            "w1": np.ascontiguousarray(np.asarray(w1[e], f32)).astype(
                ml_dtypes.bfloat16
            ),
            "w2": np.ascontiguousarray(np.asarray(w2[e], f32)).astype(
                ml_dtypes.bfloat16
            ),
            "rw": rw_r,
            "sw1": sw1s,
            "sw2": sw2s,
            "b1v": np.ascontiguousarray(np.asarray(b1[e], f32).reshape(NH, 128).T),
            "sb1v": sb1v,
            "ident": ident,
            "lx": lxm,
            "io16": io16,
            "eoffs": eoffs,
            "ones1": ones1,
            "onec": onec,
        }
        if has_rb:
            m["rbr"] = np.tile(np.asarray(router_b, f32), (128, 1))
        if has_b2:
            m["b2r"] = np.tile(np.asarray(b2[e], f32), (128, 1))
        if has_sb2:
            m["sb2r"] = np.tile(sb2_eff, (128, 1))
        in_maps.append(m)
    return key, in_maps


def _prep_dense(x, router_w, router_b, w1, b1, w2, b2, sw1, sb1, sw2, sb2):
    f32 = np.float32
    x2 = np.ascontiguousarray(np.asarray(x, f32).reshape(N, D))
    xt = np.ascontiguousarray(x2.T)
    key, sb2_eff = _flags(router_b, b2, sb2)
    has_rb, has_b2, has_sb2 = key

    rw_r = np.ascontiguousarray(
        np.asarray(router_w, f32).reshape(ND, 128, E).transpose(1, 0, 2)
    )
    ident = np.eye(128, dtype=f32)
    sw1s = np.ascontiguousarray(np.asarray(sw1, f32).reshape(S * D, HE))
    sw2s = np.ascontiguousarray(
        (np.asarray(sw2, f32) * (1.0 / S)).reshape(S * HE, D)
    ).astype(ml_dtypes.bfloat16)
    sb1v = np.ascontiguousarray(
        np.asarray(sb1, f32).reshape(S, NH, 128).transpose(2, 0, 1).reshape(128, S * NH)
    )

    in_maps = []
    for e in range(NCORES):
        m = {
            "xt": xt,
            "xsl": np.ascontiguousarray(xt[:, e * NT : (e + 1) * NT]),
            "w1": np.ascontiguousarray(np.asarray(w1[e], f32)),
            "w2": np.ascontiguousarray(np.asarray(w2[e], f32)).astype(
                ml_dtypes.bfloat16
            ),
            "rw": rw_r,
            "sw1": sw1s,
            "sw2": sw2s,
            "b1v": np.ascontiguousarray(np.asarray(b1[e], f32).reshape(NH, 128).T),
            "sb1v": sb1v,
            "ident": ident,
        }
        if has_rb:
            m["rbr"] = np.tile(np.asarray(router_b, f32), (128, 1))
        if has_b2:
            m["b2r"] = np.tile(np.asarray(b2[e], f32), (128, 1))
        if has_sb2:
            m["sb2r"] = np.tile(sb2_eff, (128, 1))
        in_maps.append(m)
    return key, in_maps


def _install_ntff_hook():
    """Re-create the boot-time NTFF profile hook (this image's antenv lacks
    axon_hooks, so trn_boot degraded silently). Needed only for tracing."""
    import contextlib
    import ctypes
    import types

    try:
        from antenv.axon_hooks import get_axon_ntff_profile_hook  # noqa: F401

        return
    except ImportError:
        pass

    so_path = "/opt/axon/libaxon_pjrt.so"
    lib = ctypes.CDLL(so_path)
    if not hasattr(lib, "axon_start_nrt_profile"):
        return
    lib.axon_start_nrt_profile.argtypes = [
        ctypes.POINTER(ctypes.c_int64),
        ctypes.c_size_t,
    ]
    lib.axon_start_nrt_profile.restype = ctypes.c_int64
    lib.axon_stop_nrt_profile.argtypes = [ctypes.c_char_p]
    lib.axon_stop_nrt_profile.restype = ctypes.c_int64

    @contextlib.contextmanager
    def _hook(output_dir, device_ids):
        import jax

        jax.devices()
        if device_ids:
            ids = (ctypes.c_int64 * len(device_ids))(*device_ids)
            rc = lib.axon_start_nrt_profile(ids, len(device_ids))
        else:
            rc = lib.axon_start_nrt_profile(None, 0)
        if rc != 0:
            raise RuntimeError(f"axon_start_nrt_profile rc={rc}")
        try:
            yield
        finally:
            n = lib.axon_stop_nrt_profile(str(output_dir).encode())
            print(f"profile: {n} file(s) written to {output_dir}", file=sys.stderr)

    mod = types.ModuleType("antenv.axon_hooks")
    mod.get_axon_ntff_profile_hook = lambda: _hook
    mod.set_axon_ntff_profile_hook = lambda h: None
    sys.modules["antenv.axon_hooks"] = mod


def kernel(x, router_w, router_b, w1, b1, w2, b2, sw1, sb1, sw2, sb2, _trace=False):
    if _trace:
        _install_ntff_hook()
    args = (x, router_w, router_b, w1, b1, w2, b2, sw1, sb1, sw2, sb2)
    key, in_maps = _prep_v2(*args)
    nc = _get_nc_v2(key)
    res = run_bass_kernel_spmd(nc, in_maps, core_ids=list(range(NCORES)), trace=_trace)
    counts = [int(c) for e in range(NCORES) for c in res.results[e]["nf_out"]]
    out = np.empty((N, D), np.float32)
    if max(counts) > CAPR:
        # capacity overflow (pathologically imbalanced routing):
        # fall back to the dense variant, correct for any routing
        key, in_maps = _prep_dense(*args)
        nc = _get_nc_dense(key)
        res = run_bass_kernel_spmd(
            nc, in_maps, core_ids=list(range(NCORES)), trace=_trace
        )
        for e in range(NCORES):
            out[e * NT : (e + 1) * NT] = res.results[e]["y_out"]
    else:
        for e in range(NCORES):
            yo = res.results[e]["y_out"]
            for q in range(4):
                out[(q * 8 + e) * 256 : (q * 8 + e + 1) * 256] = yo[
                    q * 256 : (q + 1) * 256
                ]
    out = out.reshape(np.asarray(x).shape)
    if _trace:
        return out, res
    return out
